# revision 24
# baseline (speedup 1.0000x reference)
"""Trainium2 Bass kernel for nn_Attn_40046275068166.

Tiny causal MHA over huge batch: x[B=65536, T=34, D=6], 2 heads, head_dim 3.
Pure data parallelism over 8 cores (batch sharded), batch on the 128 SBUF
partitions inside each core, G=4 examples per partition per tile.

v3 design:
- All score/exp/PV/output-projection elementwise work in bf16; DVE
  tensor_tensor ops with every operand 2-byte + inner-stride-1 run in the
  2x perf mode. Reduces keep fp32 outputs (accuracy) and run 1x.
- The score outer-product's broadcast operand xp[i,a] (stride-0 along j,
  which disqualifies 2x) is materialized into packed bf16 planes xbc by
  the otherwise-idle ACT engine (Copy activation), making the score muls
  2x-eligible.
- exp runs on ACT in-place over the causal blocks only; dead quarter of
  the score plane is never written or read (no memsets).
- Projections (y = A_h xp, v = Wv xt; fp32 accumulate, bf16 store) run on
  the GPSIMD engine, overlapped with DVE work.
- Engines per tile n: GPSIMD proj(n)+DMA, ACT xbc(n)+exp(n-1),
  DVE phase_a(n) [scores] + phase_b(n-1) [softmax+PV+outproj].

Math identity: s[b,h,i,j] = xp_i^T A_h xp_j with A_h = Wq_h^T Wk_h/sqrt(hd),
so only y = A_h xp and v = Wv xt are projected and s = xp_i . y_j.
Causal mask applied additively (-1e9, bf16) on the two diagonal blocks.

Raw bass with explicit semaphores - this walrus build allows at most one
sync-wait per instruction, so multi-dependencies are standalone wait ops.
"""

import math
from contextlib import ExitStack
from functools import lru_cache

import numpy as np

import concourse.bass as bass
from concourse import mybir
from concourse.bass_utils import run_bass_kernel_spmd

NCORES = 8
T = 34
D = 6
NH = 2
HD = 3
POS = 3
TT = T * T          # 1156
STT = NH * TT
P = 128

F32 = mybir.dt.float32
BF16 = mybir.dt.bfloat16

# fp32 constants (projection weights)
OFF_A2 = 0          # [2][6][3]  w=0: y-proj weights, w=1: v-proj weights
CLEN = 36
# bf16 constants
OFF_MASK = 0        # [1156]     additive causal mask (0 / -1e9)
OFF_WO = TT         # [6][6]     WoM[dm][e]
CBLEN = TT + 36


def _ap(t, off, dims):
    p0 = t[:].ap[0]
    return bass.AP(tensor=t, offset=off, ap=[list(p0)] + [list(d) for d in dims])


def build_kernel(bc, G):
    assert bc % (P * G) == 0
    NT = bc // (P * G)

    nc = bass.Bass("TRN2")
    x = nc.dram_tensor("x", [bc, T, D], F32, kind="ExternalInput")
    wts = nc.dram_tensor("wts", [CLEN], F32, kind="ExternalInput")
    wtsb = nc.dram_tensor("wtsb", [CBLEN], BF16, kind="ExternalInput")
    out = nc.dram_tensor("out", [bc, T, D], F32, kind="ExternalOutput")

    xr = x[:].rearrange("(n g p) t d -> n p g t d", g=G, p=P)
    outr = out[:].rearrange("(n g p) t d -> n p g t d", g=G, p=P)
    wts_b = bass.AP(tensor=wts, offset=0, ap=[[0, P], [1, CLEN]])
    wtsb_b = bass.AP(tensor=wtsb, offset=0, ap=[[0, P], [1, CBLEN]])

    with ExitStack() as ctx:
        sb = lambda nm, shape, dt=F32: ctx.enter_context(
            nc.sbuf_tensor(nm, shape, dt))
        wsb = sb("wsb", [P, CLEN])
        wsbb = sb("wsbb", [P, CBLEN], BF16)
        xin = sb("xin", [P, 2, G, T, D])
        yv = sb("yv", [P, 3, 2, G, D, T], BF16)   # [n%3][w][g][hc][j]
        pp = sb("pp", [P, G, NH, T, T], BF16)
        t1 = sb("t1", [P, 2, G, NH, T, T], BF16)
        scr = sb("scr", [P, G, NH, T, T], BF16)   # vector-private
        xbc = sb("xbc", [P, POS, G, T, T], BF16)  # ACT-written xp planes
        den = sb("den", [P, G, NH, T])
        rcp = sb("rcp", [P, G, NH, T])
        o2 = sb("o2", [P, G, T, D])
        o2b = sb("o2b", [P, G, T, D], BF16)
        prod = sb("prod", [P, G, T, D, D], BF16)  # [(g,t)][dm][e]
        p3 = sb("p3", [P, G, T, D, HD], BF16)
        res = sb("res", [P, 2, G, T, D])
        pacc = sb("pacc", [P, G, D, T])           # gpsimd-private
        ptmp = sb("ptmp", [P, G, D, T])
        # bf16 fold-tree scratches (vector-private, reused per reduction)
        f17 = sb("f17", [P, G, NH, 17, 17], BF16)
        tw8 = sb("tw8", [P, G, NH, T, 8], BF16)
        tw4 = sb("tw4", [P, G, NH, T, 4], BF16)
        tw2 = sb("tw2", [P, G, NH, T, 2], BF16)
        tsum = sb("tsum", [P, G, NH, T], BF16)
        # gpsimd-private den-tree scratches
        qf17 = sb("qf17", [P, G, NH, 17, 17], BF16)
        qw8 = sb("qw8", [P, G, NH, T, 8], BF16)
        qw4 = sb("qw4", [P, G, NH, T, 4], BF16)
        qw2 = sb("qw2", [P, G, NH, T, 2], BF16)
        qsum = sb("qsum", [P, G, NH, T], BF16)

        sem_names = ["dma_in0", "dma_in1", "const", "constb", "proj_done",
                     "a_done", "s_done", "e_done", "b_done", "res_done",
                     "d_done", "out0", "out1"]
        sems = {k: ctx.enter_context(nc.semaphore(name=k)) for k in sem_names}

        XIN_SET = G * T * D
        XIN_G = T * D
        YV_BUF = 2 * G * T * D
        YV_W = G * T * D
        YV_G = T * D
        TS_SET = G * NH * TT
        H = T // 2
        BLKS = [(0, 0), (H, 0), (H, H)]

        block = ctx.enter_context(nc.Block())

        @block.gpsimd
        def _(sync):
            def store(k):
                sp = k % 2
                sync.wait_ge(sems["res_done"], k + 1)
                sync.dma_start(
                    out=outr[k],
                    in_=_ap(res, sp * XIN_SET, [(XIN_G, G), (1, T * D)]),
                ).then_inc(sems["out0" if sp == 0 else "out1"], 16)

            def load(n):
                s = n % 2
                if n >= 2:
                    # xin[s] free once ACT's xbc(n-2) has read it
                    sync.wait_ge(sems["a_done"], n - 1)
                sync.dma_start(
                    out=_ap(xin, s * XIN_SET, [(XIN_G, G), (1, T * D)]),
                    in_=xr[n],
                ).then_inc(sems["dma_in0" if s == 0 else "dma_in1"], 16)

            def proj(n):
                s = n % 2
                m = n % 3
                sync.wait_ge(sems["dma_in0" if s == 0 else "dma_in1"],
                             16 * (n // 2 + 1))
                if n >= 3:
                    # WAR: yv[m] last read by PV muls of phase_b(n-3)
                    sync.wait_ge(sems["b_done"], n - 2)
                last = None
                for w in range(2):
                    xoff = s * XIN_SET + (3 - 3 * w)
                    for b in range(POS):
                        i0 = _ap(xin, xoff + b, [(XIN_G, G), (0, D), (D, T)])
                        i1 = _ap(wsb, OFF_A2 + w * 18 + b,
                                 [(0, G), (3, D), (0, T)])
                        if b == 0:
                            sync.tensor_mul(
                                out=_ap(pacc, 0, [(D * T, G), (T, D), (1, T)]),
                                in0=i0, in1=i1)
                        else:
                            sync.tensor_mul(
                                out=_ap(ptmp, 0, [(D * T, G), (T, D), (1, T)]),
                                in0=i0, in1=i1)
                            dst = (_ap(yv, m * YV_BUF + w * YV_W,
                                       [(YV_G, G), (1, T * D)]) if b == 2
                                   else _ap(pacc, 0, [(D * T, G), (1, T * D)]))
                            last = sync.tensor_add(
                                out=dst,
                                in0=_ap(pacc, 0, [(D * T, G), (1, T * D)]),
                                in1=_ap(ptmp, 0, [(D * T, G), (1, T * D)]))
                last.then_inc(sems["proj_done"], 1)

            GH = G * NH

            def dtree(n):
                """den row-sums of t1[s] via bf16 fold tree, on Pool."""
                s = n % 2
                soff = s * TS_SET
                sync.wait_ge(sems["e_done"], n + 1)
                sync.tensor_add(
                    out=_ap(qf17, 0, [(289, GH), (17, 17), (1, 17)]),
                    in0=_ap(t1, soff + H * T, [(TT, GH), (T, 17), (1, 17)]),
                    in1=_ap(t1, soff + H * T + H,
                            [(TT, GH), (T, 17), (1, 17)]))
                sync.tensor_add(
                    out=_ap(qw8, 0, [(T * 8, GH), (8, 17), (1, 8)]),
                    in0=_ap(t1, soff, [(TT, GH), (T, 17), (1, 8)]),
                    in1=_ap(t1, soff + 8, [(TT, GH), (T, 17), (1, 8)]))
                sync.tensor_add(
                    out=_ap(qw8, 17 * 8, [(T * 8, GH), (8, 17), (1, 8)]),
                    in0=_ap(qf17, 0, [(289, GH), (17, 17), (1, 8)]),
                    in1=_ap(qf17, 8, [(289, GH), (17, 17), (1, 8)]))
                sync.tensor_add(
                    out=_ap(qw4, 0, [(T * 4, GH), (4, T), (1, 4)]),
                    in0=_ap(qw8, 0, [(T * 8, GH), (8, T), (1, 4)]),
                    in1=_ap(qw8, 4, [(T * 8, GH), (8, T), (1, 4)]))
                sync.tensor_add(
                    out=_ap(qw2, 0, [(T * 2, GH), (2, T), (1, 2)]),
                    in0=_ap(qw4, 0, [(T * 4, GH), (4, T), (1, 2)]),
                    in1=_ap(qw4, 2, [(T * 4, GH), (4, T), (1, 2)]))
                sync.tensor_add(
                    out=_ap(qsum, 0, [(T, GH), (1, T)]),
                    in0=_ap(qw2, 0, [(T * 2, GH), (2, T)]),
                    in1=_ap(qw2, 1, [(T * 2, GH), (2, T)]))
                sync.tensor_add(
                    out=_ap(den, 0, [(T, GH), (1, 17)]),
                    in0=_ap(qsum, 0, [(T, GH), (1, 17)]),
                    in1=_ap(t1, soff + 16, [(TT, GH), (T, 17)]))
                sync.tensor_add(
                    out=_ap(den, H, [(T, GH), (1, 17)]),
                    in0=_ap(qsum, 17, [(T, GH), (1, 17)]),
                    in1=_ap(qf17, 16, [(289, GH), (17, 17)])
                ).then_inc(sems["d_done"], 1)

            sync.dma_start(out=wsb[:], in_=wts_b).then_inc(sems["const"], 16)
            sync.dma_start(out=wsbb[:], in_=wtsb_b).then_inc(
                sems["constb"], 16)
            load(0)
            load(1)
            sync.wait_ge(sems["const"], 16)
            for n in range(NT):
                proj(n)
                if n >= 2:
                    store(n - 2)
                if n >= 1:
                    dtree(n - 1)
                if n + 2 < NT:
                    load(n + 2)
            dtree(NT - 1)
            store(NT - 2)
            store(NT - 1)
            sync.wait_ge(sems["out0"], 16 * ((NT + 1) // 2))
            sync.wait_ge(sems["out1"], 16 * (NT // 2))

        @block.scalar
        def _(scalar):
            def xbc_fill(n):
                # xbc[a, g, i, j] = xp[g, i, a]  (bf16, j-packed), causal
                # blocks only: AB column (all rows, j<H) + C diag block.
                s = n % 2
                scalar.wait_ge(sems["dma_in0" if s == 0 else "dma_in1"],
                               16 * (n // 2 + 1))
                last = None
                for a in range(POS):
                    for (ro, io, li, lj) in ((0, 0, T, H),
                                             (H * T + H, H * D, T - H, T - H)):
                        last = scalar.activation(
                            out=_ap(xbc, a * G * TT + ro,
                                    [(TT, G), (T, li), (1, lj)]),
                            in_=_ap(xin, s * XIN_SET + 3 + a + io,
                                    [(XIN_G, G), (D, li), (0, lj)]),
                            func=mybir.ActivationFunctionType.Copy,
                        )
                last.then_inc(sems["a_done"], 1)

            xbc_fill(0)
            for n in range(NT):
                s = n % 2
                if n >= 2:
                    scalar.wait_ge(sems["b_done"], n - 1)
                    scalar.wait_ge(sems["d_done"], n - 1)
                scalar.wait_ge(sems["s_done"], n + 1)
                # exp in-place over causal blocks of t1[s]
                scalar.activation(
                    out=_ap(t1, s * TS_SET, [(TT, G * NH), (T, T), (1, H)]),
                    in_=_ap(t1, s * TS_SET, [(TT, G * NH), (T, T), (1, H)]),
                    func=mybir.ActivationFunctionType.Exp,
                )
                ro = H * T + H
                scalar.activation(
                    out=_ap(t1, s * TS_SET + ro,
                            [(TT, G * NH), (T, T - H), (1, T - H)]),
                    in_=_ap(t1, s * TS_SET + ro,
                            [(TT, G * NH), (T, T - H), (1, T - H)]),
                    func=mybir.ActivationFunctionType.Exp,
                ).then_inc(sems["e_done"], 1)
                if n + 1 < NT:
                    xbc_fill(n + 1)

        @block.vector
        def _(vector):
            vector.wait_ge(sems["constb"], 16)

            def phase_a(n):
                s = n % 2
                m = n % 3
                vector.wait_ge(sems["proj_done"], n + 1)
                vector.wait_ge(sems["a_done"], n + 1)
                if n >= 2:
                    # t1[s] still read by Pool's dtree(n-2)
                    vector.wait_ge(sems["d_done"], n - 1)
                # scores: t1[g,h,i,j] = sum_a xbc[a,g,i,j] * y[g,(h,a),j]
                # two merged causal regions: AB column (all rows, j<H) and
                # C diag block
                for a in range(POS):
                    dst = t1 if a == 0 else scr
                    doff = s * TS_SET if a == 0 else 0
                    for h in range(NH):
                        for (ro, li) in ((0, T), (H * T + H, T - H)):
                            vector.tensor_mul(
                                out=_ap(dst, doff + h * TT + ro,
                                        [(NH * TT, G), (T, li), (1, H)]),
                                in0=_ap(xbc, a * G * TT + ro,
                                        [(TT, G), (T, li), (1, H)]),
                                in1=_ap(yv, m * YV_BUF + (h * HD + a) * T
                                        + (0 if ro == 0 else H),
                                        [(YV_G, G), (0, li), (1, H)]))
                    if a == 2:
                        # scr += mask on diagonal blocks A and C
                        for ro in (0, H * T + H):
                            vector.tensor_add(
                                out=_ap(scr, ro, [(TT, G * NH), (T, H), (1, H)]),
                                in0=_ap(scr, ro, [(TT, G * NH), (T, H), (1, H)]),
                                in1=_ap(wsbb, OFF_MASK + ro,
                                        [(0, G * NH), (T, H), (1, H)]))
                    if a > 0:
                        # t1 += scr over AB column + C block
                        mm = None
                        for (ro, li, lj) in ((0, T, H), (H * T + H, T - H, T - H)):
                            mm = vector.tensor_add(
                                out=_ap(t1, s * TS_SET + ro,
                                        [(TT, G * NH), (T, li), (1, lj)]),
                                in0=_ap(t1, s * TS_SET + ro,
                                        [(TT, G * NH), (T, li), (1, lj)]),
                                in1=_ap(scr, ro,
                                        [(TT, G * NH), (T, li), (1, lj)]))
                        if a == 2:
                            mm.then_inc(sems["s_done"], 1)

            GH = G * NH

            def fold_tree(src, soff):
                """bf16 pairwise row-sum tree over causal planes of src.

                After this: tsum[g,h,i] + leftover col 16 (A rows: src[i,16],
                BC rows: f17[i-17,16]) equals the causal row sum."""
                # L1: rows 17-33: fold j and j+17
                vector.tensor_add(
                    out=_ap(f17, 0, [(289, GH), (17, 17), (1, 17)]),
                    in0=_ap(src, soff + H * T, [(TT, GH), (T, 17), (1, 17)]),
                    in1=_ap(src, soff + H * T + H,
                            [(TT, GH), (T, 17), (1, 17)]))
                # L2a: A rows (0-16): j pairs (0-7)+(8-15)
                vector.tensor_add(
                    out=_ap(tw8, 0, [(T * 8, GH), (8, 17), (1, 8)]),
                    in0=_ap(src, soff, [(TT, GH), (T, 17), (1, 8)]),
                    in1=_ap(src, soff + 8, [(TT, GH), (T, 17), (1, 8)]))
                # L2f: folded rows -> tw8 rows 17-33
                vector.tensor_add(
                    out=_ap(tw8, 17 * 8, [(T * 8, GH), (8, 17), (1, 8)]),
                    in0=_ap(f17, 0, [(289, GH), (17, 17), (1, 8)]),
                    in1=_ap(f17, 8, [(289, GH), (17, 17), (1, 8)]))
                # L3: 8->4, L4: 4->2
                vector.tensor_add(
                    out=_ap(tw4, 0, [(T * 4, GH), (4, T), (1, 4)]),
                    in0=_ap(tw8, 0, [(T * 8, GH), (8, T), (1, 4)]),
                    in1=_ap(tw8, 4, [(T * 8, GH), (8, T), (1, 4)]))
                vector.tensor_add(
                    out=_ap(tw2, 0, [(T * 2, GH), (2, T), (1, 2)]),
                    in0=_ap(tw4, 0, [(T * 4, GH), (4, T), (1, 2)]),
                    in1=_ap(tw4, 2, [(T * 4, GH), (4, T), (1, 2)]))
                # L5: 2->1
                vector.tensor_add(
                    out=_ap(tsum, 0, [(T, GH), (1, T)]),
                    in0=_ap(tw2, 0, [(T * 2, GH), (2, T)]),
                    in1=_ap(tw2, 1, [(T * 2, GH), (2, T)]))

            def phase_b(n):
                s = n % 2
                m = n % 3
                vector.wait_ge(sems["e_done"], n + 1)
                if n >= 2:
                    vector.wait_ge(sems["out0" if s == 0 else "out1"],
                                   16 * (n // 2))
                for c in range(HD):
                    for h in range(NH):
                        for bi, (ro, li) in enumerate(((0, T),
                                                      (H * T + H, T - H))):
                            mm = vector.tensor_mul(
                                out=_ap(pp, h * TT + ro,
                                        [(NH * TT, G), (T, li), (1, H)]),
                                in0=_ap(t1, s * TS_SET + h * TT + ro,
                                        [(NH * TT, G), (T, li), (1, H)]),
                                in1=_ap(yv, m * YV_BUF + YV_W + (h * HD + c) * T
                                        + (0 if ro == 0 else H),
                                        [(YV_G, G), (0, li), (1, H)]))
                            if c == HD - 1 and h == NH - 1 and bi == 1:
                                mm.then_inc(sems["b_done"], 1)
                    fold_tree(pp, 0)
                    vector.tensor_add(
                        out=_ap(o2, c, [(T * D, G), (HD, NH), (D, 17)]),
                        in0=_ap(tsum, 0, [(NH * T, G), (T, NH), (1, 17)]),
                        in1=_ap(pp, 16, [(NH * TT, G), (TT, NH), (T, 17)]))
                    vector.tensor_add(
                        out=_ap(o2, c + H * D, [(T * D, G), (HD, NH), (D, 17)]),
                        in0=_ap(tsum, 17, [(NH * T, G), (T, NH), (1, 17)]),
                        in1=_ap(f17, 16, [(NH * 289, G), (289, NH), (17, 17)]))
                # den computed on Pool; reciprocal + normalize here
                vector.wait_ge(sems["d_done"], n + 1)
                vector.reciprocal(
                    out=_ap(rcp, 0, [(1, G * NH * T)]),
                    in_=_ap(den, 0, [(1, G * NH * T)]))
                for g in range(G):
                    vector.tensor_mul(
                        out=_ap(o2b, g * T * D, [(D, T), (HD, NH), (1, HD)]),
                        in0=_ap(o2, g * T * D, [(D, T), (HD, NH), (1, HD)]),
                        in1=_ap(rcp, g * NH * T, [(1, T), (T, NH), (0, HD)]))
                # outproj: prod[(g,t),dm,e] = o2b[g,t,e] * Wo[dm,e], fold
                # e-halves, reduce remaining 3
                vector.tensor_mul(
                    out=_ap(prod, 0, [(D * D, G * T), (D, D), (1, D)]),
                    in0=_ap(o2b, 0, [(D, G * T), (0, D), (1, D)]),
                    in1=_ap(wsbb, OFF_WO, [(0, G * T), (D, D), (1, D)]))
                vector.tensor_add(
                    out=_ap(p3, 0, [(D * HD, G * T), (HD, D), (1, HD)]),
                    in0=_ap(prod, 0, [(D * D, G * T), (D, D), (1, HD)]),
                    in1=_ap(prod, HD, [(D * D, G * T), (D, D), (1, HD)]))
                vector.tensor_reduce(
                    out=_ap(res, s * XIN_SET, [(D, G * T), (1, D)]),
                    in_=_ap(p3, 0, [(D * HD, G * T), (HD, D), (1, HD)]),
                    axis=mybir.AxisListType.X, op=mybir.AluOpType.add
                ).then_inc(sems["res_done"], 1)

            for n in range(NT):
                phase_a(n)
                if n >= 1:
                    phase_b(n - 1)
            phase_b(NT - 1)

    return nc


def _pack_weights(Wq, Wk, Wv, Wo):
    from ml_dtypes import bfloat16
    wts = np.zeros(CLEN, dtype=np.float32)
    scale = 1.0 / math.sqrt(HD)
    A2 = wts[OFF_A2:OFF_A2 + 36].reshape(2, D, POS)
    for h in range(NH):
        A2[0, h * HD:(h + 1) * HD, :] = (Wq[h * HD:(h + 1) * HD, :].T
                                         @ Wk[h * HD:(h + 1) * HD, :]) * scale
        A2[1, h * HD:(h + 1) * HD, :] = Wv[h * HD:(h + 1) * HD, :]
    wtsb = np.zeros(CBLEN, dtype=np.float32)
    mask = np.where(np.tril(np.ones((T, T))) > 0, 0.0, -1e9).astype(np.float32)
    wtsb[OFF_MASK:OFF_MASK + TT] = mask.reshape(-1)
    wtsb[OFF_WO:OFF_WO + 36] = Wo.reshape(-1)
    return wts, wtsb.astype(bfloat16)


@lru_cache(maxsize=2)
def _cached_kernel(bc, G):
    return build_kernel(bc, G)


def kernel(x, Wq, Wk, Wv, Wo):
    x = np.ascontiguousarray(x, dtype=np.float32)
    B = x.shape[0]
    bc = B // NCORES
    G = 4
    nc = _cached_kernel(bc, G)
    wts, wtsb = _pack_weights(np.asarray(Wq, dtype=np.float32),
                              np.asarray(Wk, dtype=np.float32),
                              np.asarray(Wv, dtype=np.float32),
                              np.asarray(Wo, dtype=np.float32))
    in_maps = [{"x": x[i * bc:(i + 1) * bc], "wts": wts, "wtsb": wtsb}
               for i in range(NCORES)]
    r = run_bass_kernel_spmd(nc, in_maps, core_ids=list(range(NCORES)))
    return np.concatenate([m["out"] for m in r.results], axis=0)


# revision 25
# speedup vs baseline: 1.1261x; 1.1261x over previous
"""Trainium2 Bass kernel for nn_Attn_40046275068166.

Tiny causal MHA over huge batch: x[B=65536, T=34, D=6], 2 heads, head_dim 3.
Pure data parallelism over 8 cores (batch sharded), batch on the 128 SBUF
partitions inside each core, G=4 examples per partition per tile.

v3 design:
- All score/exp/PV/output-projection elementwise work in bf16; DVE
  tensor_tensor ops with every operand 2-byte + inner-stride-1 run in the
  2x perf mode. Reduces keep fp32 outputs (accuracy) and run 1x.
- The score outer-product's broadcast operand xp[i,a] (stride-0 along j,
  which disqualifies 2x) is materialized into packed bf16 planes xbc by
  the otherwise-idle ACT engine (Copy activation), making the score muls
  2x-eligible.
- exp runs on ACT in-place over the causal blocks only; dead quarter of
  the score plane is never written or read (no memsets).
- Projections (y = A_h xp, v = Wv xt; fp32 accumulate, bf16 store) run on
  the GPSIMD engine, overlapped with DVE work.
- Engines per tile n: GPSIMD proj(n)+DMA, ACT xbc(n)+exp(n-1),
  DVE phase_a(n) [scores] + phase_b(n-1) [softmax+PV+outproj].

Math identity: s[b,h,i,j] = xp_i^T A_h xp_j with A_h = Wq_h^T Wk_h/sqrt(hd),
so only y = A_h xp and v = Wv xt are projected and s = xp_i . y_j.
Causal mask applied additively (-1e9, bf16) on the two diagonal blocks.

Raw bass with explicit semaphores - this walrus build allows at most one
sync-wait per instruction, so multi-dependencies are standalone wait ops.
"""

import math
from contextlib import ExitStack
from functools import lru_cache

import numpy as np

import concourse.bass as bass
from concourse import mybir
from concourse.bass_utils import run_bass_kernel_spmd

NCORES = 8
T = 34
D = 6
NH = 2
HD = 3
POS = 3
TT = T * T          # 1156
STT = NH * TT
P = 128

F32 = mybir.dt.float32
BF16 = mybir.dt.bfloat16

# fp32 constants (projection weights)
OFF_A2 = 0          # [2][6][3]  w=0: y-proj weights, w=1: v-proj weights
CLEN = 36
# bf16 constants
OFF_MASK = 0        # [1156]     additive causal mask (0 / -1e9)
OFF_WO = TT         # [6][6]     WoM[dm][e]
CBLEN = TT + 36


def _ap(t, off, dims):
    p0 = t[:].ap[0]
    return bass.AP(tensor=t, offset=off, ap=[list(p0)] + [list(d) for d in dims])


def build_kernel(bc, G):
    assert bc % (P * G) == 0
    NT = bc // (P * G)

    nc = bass.Bass("TRN2")
    x = nc.dram_tensor("x", [bc, T, D], F32, kind="ExternalInput")
    wts = nc.dram_tensor("wts", [CLEN], F32, kind="ExternalInput")
    wtsb = nc.dram_tensor("wtsb", [CBLEN], BF16, kind="ExternalInput")
    out = nc.dram_tensor("out", [bc, T, D], F32, kind="ExternalOutput")

    xr = x[:].rearrange("(n g p) t d -> n p g t d", g=G, p=P)
    outr = out[:].rearrange("(n g p) t d -> n p g t d", g=G, p=P)
    wts_b = bass.AP(tensor=wts, offset=0, ap=[[0, P], [1, CLEN]])
    wtsb_b = bass.AP(tensor=wtsb, offset=0, ap=[[0, P], [1, CBLEN]])

    with ExitStack() as ctx:
        sb = lambda nm, shape, dt=F32: ctx.enter_context(
            nc.sbuf_tensor(nm, shape, dt))
        wsb = sb("wsb", [P, CLEN])
        wsbb = sb("wsbb", [P, CBLEN], BF16)
        xin = sb("xin", [P, 2, G, T, D])
        yv = sb("yv", [P, 3, 2, G, D, T], BF16)   # [n%3][w][g][hc][j]
        pp = sb("pp", [P, G, NH, T, T], BF16)
        t1 = sb("t1", [P, 2, G, NH, T, T], BF16)
        scr = sb("scr", [P, G, NH, T, T], BF16)   # vector-private
        xbc = sb("xbc", [P, POS, G, T, T], BF16)  # ACT-written xp planes
        den = sb("den", [P, G, NH, T])
        rcp = sb("rcp", [P, G, NH, T])
        o2 = sb("o2", [P, G, T, D])
        o2b = sb("o2b", [P, G, T, D], BF16)
        prod = sb("prod", [P, G, T, D, D], BF16)  # [(g,t)][dm][e]
        p3 = sb("p3", [P, G, T, D, HD], BF16)
        res = sb("res", [P, 2, G, T, D])
        pacc = sb("pacc", [P, G, D, T])           # gpsimd-private
        ptmp = sb("ptmp", [P, G, D, T])
        # bf16 fold-tree scratches (vector-private, reused per reduction)
        f17 = sb("f17", [P, G, NH, 17, 17], BF16)
        tw8 = sb("tw8", [P, G, NH, T, 8], BF16)
        tw4 = sb("tw4", [P, G, NH, T, 4], BF16)
        tw2 = sb("tw2", [P, G, NH, T, 2], BF16)
        tsum = sb("tsum", [P, G, NH, T], BF16)

        sem_names = ["dma_in0", "dma_in1", "const", "constb", "proj_done",
                     "a_done", "s_done", "e_done", "b_done", "res_done",
                     "d_done", "out0", "out1"]
        sems = {k: ctx.enter_context(nc.semaphore(name=k)) for k in sem_names}

        XIN_SET = G * T * D
        XIN_G = T * D
        YV_BUF = 2 * G * T * D
        YV_W = G * T * D
        YV_G = T * D
        TS_SET = G * NH * TT
        H = T // 2
        BLKS = [(0, 0), (H, 0), (H, H)]

        block = ctx.enter_context(nc.Block())

        @block.gpsimd
        def _(sync):
            def store(k):
                sp = k % 2
                sync.wait_ge(sems["res_done"], k + 1)
                sync.dma_start(
                    out=outr[k],
                    in_=_ap(res, sp * XIN_SET, [(XIN_G, G), (1, T * D)]),
                ).then_inc(sems["out0" if sp == 0 else "out1"], 16)

            def load(n):
                s = n % 2
                if n >= 2:
                    # xin[s] free once ACT's xbc(n-2) has read it
                    sync.wait_ge(sems["a_done"], n - 1)
                sync.dma_start(
                    out=_ap(xin, s * XIN_SET, [(XIN_G, G), (1, T * D)]),
                    in_=xr[n],
                ).then_inc(sems["dma_in0" if s == 0 else "dma_in1"], 16)

            def proj(n):
                s = n % 2
                m = n % 3
                sync.wait_ge(sems["dma_in0" if s == 0 else "dma_in1"],
                             16 * (n // 2 + 1))
                if n >= 3:
                    # WAR: yv[m] last read by PV muls of phase_b(n-3)
                    sync.wait_ge(sems["b_done"], n - 2)
                last = None
                for w in range(2):
                    xoff = s * XIN_SET + (3 - 3 * w)
                    for b in range(POS):
                        i0 = _ap(xin, xoff + b, [(XIN_G, G), (0, D), (D, T)])
                        i1 = _ap(wsb, OFF_A2 + w * 18 + b,
                                 [(0, G), (3, D), (0, T)])
                        if b == 0:
                            sync.tensor_mul(
                                out=_ap(pacc, 0, [(D * T, G), (T, D), (1, T)]),
                                in0=i0, in1=i1)
                        else:
                            sync.tensor_mul(
                                out=_ap(ptmp, 0, [(D * T, G), (T, D), (1, T)]),
                                in0=i0, in1=i1)
                            dst = (_ap(yv, m * YV_BUF + w * YV_W,
                                       [(YV_G, G), (1, T * D)]) if b == 2
                                   else _ap(pacc, 0, [(D * T, G), (1, T * D)]))
                            last = sync.tensor_add(
                                out=dst,
                                in0=_ap(pacc, 0, [(D * T, G), (1, T * D)]),
                                in1=_ap(ptmp, 0, [(D * T, G), (1, T * D)]))
                last.then_inc(sems["proj_done"], 1)

            sync.dma_start(out=wsb[:], in_=wts_b).then_inc(sems["const"], 16)
            sync.dma_start(out=wsbb[:], in_=wtsb_b).then_inc(
                sems["constb"], 16)
            load(0)
            load(1)
            sync.wait_ge(sems["const"], 16)
            for n in range(NT):
                proj(n)
                if n >= 2:
                    store(n - 2)
                if n + 2 < NT:
                    load(n + 2)
            store(NT - 2)
            store(NT - 1)
            sync.wait_ge(sems["out0"], 16 * ((NT + 1) // 2))
            sync.wait_ge(sems["out1"], 16 * (NT // 2))

        @block.scalar
        def _(scalar):
            def xbc_fill(n):
                # xbc[a, g, i, j] = xp[g, i, a]  (bf16, j-packed), causal
                # blocks only: AB column (all rows, j<H) + C diag block.
                s = n % 2
                scalar.wait_ge(sems["dma_in0" if s == 0 else "dma_in1"],
                               16 * (n // 2 + 1))
                last = None
                for a in range(POS):
                    for (ro, io, li, lj) in ((0, 0, T, H),
                                             (H * T + H, H * D, T - H, T - H)):
                        last = scalar.activation(
                            out=_ap(xbc, a * G * TT + ro,
                                    [(TT, G), (T, li), (1, lj)]),
                            in_=_ap(xin, s * XIN_SET + 3 + a + io,
                                    [(XIN_G, G), (D, li), (0, lj)]),
                            func=mybir.ActivationFunctionType.Copy,
                        )
                last.then_inc(sems["a_done"], 1)

            xbc_fill(0)
            for n in range(NT):
                s = n % 2
                if n >= 2:
                    scalar.wait_ge(sems["b_done"], n - 1)
                scalar.wait_ge(sems["s_done"], n + 1)
                # exp in-place over causal blocks of t1[s]
                scalar.activation(
                    out=_ap(t1, s * TS_SET, [(TT, G * NH), (T, T), (1, H)]),
                    in_=_ap(t1, s * TS_SET, [(TT, G * NH), (T, T), (1, H)]),
                    func=mybir.ActivationFunctionType.Exp,
                )
                ro = H * T + H
                scalar.activation(
                    out=_ap(t1, s * TS_SET + ro,
                            [(TT, G * NH), (T, T - H), (1, T - H)]),
                    in_=_ap(t1, s * TS_SET + ro,
                            [(TT, G * NH), (T, T - H), (1, T - H)]),
                    func=mybir.ActivationFunctionType.Exp,
                ).then_inc(sems["e_done"], 1)
                if n + 1 < NT:
                    xbc_fill(n + 1)

        @block.vector
        def _(vector):
            vector.wait_ge(sems["constb"], 16)

            def phase_a(n):
                s = n % 2
                m = n % 3
                vector.wait_ge(sems["proj_done"], n + 1)
                vector.wait_ge(sems["a_done"], n + 1)
                # scores: t1[g,h,i,j] = sum_a xbc[a,g,i,j] * y[g,(h,a),j]
                # two merged causal regions: AB column (all rows, j<H) and
                # C diag block
                for a in range(POS):
                    dst = t1 if a == 0 else scr
                    doff = s * TS_SET if a == 0 else 0
                    for h in range(NH):
                        for (ro, li) in ((0, T), (H * T + H, T - H)):
                            vector.tensor_mul(
                                out=_ap(dst, doff + h * TT + ro,
                                        [(NH * TT, G), (T, li), (1, H)]),
                                in0=_ap(xbc, a * G * TT + ro,
                                        [(TT, G), (T, li), (1, H)]),
                                in1=_ap(yv, m * YV_BUF + (h * HD + a) * T
                                        + (0 if ro == 0 else H),
                                        [(YV_G, G), (0, li), (1, H)]))
                    if a == 2:
                        # scr += mask on diagonal blocks A and C
                        for ro in (0, H * T + H):
                            vector.tensor_add(
                                out=_ap(scr, ro, [(TT, G * NH), (T, H), (1, H)]),
                                in0=_ap(scr, ro, [(TT, G * NH), (T, H), (1, H)]),
                                in1=_ap(wsbb, OFF_MASK + ro,
                                        [(0, G * NH), (T, H), (1, H)]))
                    if a > 0:
                        # t1 += scr over AB column + C block
                        mm = None
                        for (ro, li, lj) in ((0, T, H), (H * T + H, T - H, T - H)):
                            mm = vector.tensor_add(
                                out=_ap(t1, s * TS_SET + ro,
                                        [(TT, G * NH), (T, li), (1, lj)]),
                                in0=_ap(t1, s * TS_SET + ro,
                                        [(TT, G * NH), (T, li), (1, lj)]),
                                in1=_ap(scr, ro,
                                        [(TT, G * NH), (T, li), (1, lj)]))
                        if a == 2:
                            mm.then_inc(sems["s_done"], 1)

            GH = G * NH

            def fold_tree(src, soff):
                """bf16 pairwise row-sum tree over causal planes of src.

                After this: tsum[g,h,i] + leftover col 16 (A rows: src[i,16],
                BC rows: f17[i-17,16]) equals the causal row sum."""
                # L1: rows 17-33: fold j and j+17
                vector.tensor_add(
                    out=_ap(f17, 0, [(289, GH), (17, 17), (1, 17)]),
                    in0=_ap(src, soff + H * T, [(TT, GH), (T, 17), (1, 17)]),
                    in1=_ap(src, soff + H * T + H,
                            [(TT, GH), (T, 17), (1, 17)]))
                # L2a: A rows (0-16): j pairs (0-7)+(8-15)
                vector.tensor_add(
                    out=_ap(tw8, 0, [(T * 8, GH), (8, 17), (1, 8)]),
                    in0=_ap(src, soff, [(TT, GH), (T, 17), (1, 8)]),
                    in1=_ap(src, soff + 8, [(TT, GH), (T, 17), (1, 8)]))
                # L2f: folded rows -> tw8 rows 17-33
                vector.tensor_add(
                    out=_ap(tw8, 17 * 8, [(T * 8, GH), (8, 17), (1, 8)]),
                    in0=_ap(f17, 0, [(289, GH), (17, 17), (1, 8)]),
                    in1=_ap(f17, 8, [(289, GH), (17, 17), (1, 8)]))
                # L3: 8->4, L4: 4->2
                vector.tensor_add(
                    out=_ap(tw4, 0, [(T * 4, GH), (4, T), (1, 4)]),
                    in0=_ap(tw8, 0, [(T * 8, GH), (8, T), (1, 4)]),
                    in1=_ap(tw8, 4, [(T * 8, GH), (8, T), (1, 4)]))
                vector.tensor_add(
                    out=_ap(tw2, 0, [(T * 2, GH), (2, T), (1, 2)]),
                    in0=_ap(tw4, 0, [(T * 4, GH), (4, T), (1, 2)]),
                    in1=_ap(tw4, 2, [(T * 4, GH), (4, T), (1, 2)]))
                # L5: 2->1
                vector.tensor_add(
                    out=_ap(tsum, 0, [(T, GH), (1, T)]),
                    in0=_ap(tw2, 0, [(T * 2, GH), (2, T)]),
                    in1=_ap(tw2, 1, [(T * 2, GH), (2, T)]))

            def phase_b(n):
                s = n % 2
                m = n % 3
                vector.wait_ge(sems["e_done"], n + 1)
                if n >= 2:
                    vector.wait_ge(sems["out0" if s == 0 else "out1"],
                                   16 * (n // 2))
                # den: fold tree + leftover col, fp32 out
                fold_tree(t1, s * TS_SET)
                vector.tensor_add(
                    out=_ap(den, 0, [(T, GH), (1, 17)]),
                    in0=_ap(tsum, 0, [(T, GH), (1, 17)]),
                    in1=_ap(t1, s * TS_SET + 16, [(TT, GH), (T, 17)]))
                vector.tensor_add(
                    out=_ap(den, H, [(T, GH), (1, 17)]),
                    in0=_ap(tsum, 17, [(T, GH), (1, 17)]),
                    in1=_ap(f17, 16, [(289, GH), (17, 17)]))
                vector.reciprocal(
                    out=_ap(rcp, 0, [(1, G * NH * T)]),
                    in_=_ap(den, 0, [(1, G * NH * T)]))
                for c in range(HD):
                    for h in range(NH):
                        for bi, (ro, li) in enumerate(((0, T),
                                                      (H * T + H, T - H))):
                            mm = vector.tensor_mul(
                                out=_ap(pp, h * TT + ro,
                                        [(NH * TT, G), (T, li), (1, H)]),
                                in0=_ap(t1, s * TS_SET + h * TT + ro,
                                        [(NH * TT, G), (T, li), (1, H)]),
                                in1=_ap(yv, m * YV_BUF + YV_W + (h * HD + c) * T
                                        + (0 if ro == 0 else H),
                                        [(YV_G, G), (0, li), (1, H)]))
                            if c == HD - 1 and h == NH - 1 and bi == 1:
                                mm.then_inc(sems["b_done"], 1)
                    fold_tree(pp, 0)
                    vector.tensor_add(
                        out=_ap(o2, c, [(T * D, G), (HD, NH), (D, 17)]),
                        in0=_ap(tsum, 0, [(NH * T, G), (T, NH), (1, 17)]),
                        in1=_ap(pp, 16, [(NH * TT, G), (TT, NH), (T, 17)]))
                    vector.tensor_add(
                        out=_ap(o2, c + H * D, [(T * D, G), (HD, NH), (D, 17)]),
                        in0=_ap(tsum, 17, [(NH * T, G), (T, NH), (1, 17)]),
                        in1=_ap(f17, 16, [(NH * 289, G), (289, NH), (17, 17)]))
                for g in range(G):
                    vector.tensor_mul(
                        out=_ap(o2b, g * T * D, [(D, T), (HD, NH), (1, HD)]),
                        in0=_ap(o2, g * T * D, [(D, T), (HD, NH), (1, HD)]),
                        in1=_ap(rcp, g * NH * T, [(1, T), (T, NH), (0, HD)]))
                # outproj: prod[(g,t),dm,e] = o2b[g,t,e] * Wo[dm,e], fold
                # e-halves, reduce remaining 3
                vector.tensor_mul(
                    out=_ap(prod, 0, [(D * D, G * T), (D, D), (1, D)]),
                    in0=_ap(o2b, 0, [(D, G * T), (0, D), (1, D)]),
                    in1=_ap(wsbb, OFF_WO, [(0, G * T), (D, D), (1, D)]))
                vector.tensor_add(
                    out=_ap(p3, 0, [(D * HD, G * T), (HD, D), (1, HD)]),
                    in0=_ap(prod, 0, [(D * D, G * T), (D, D), (1, HD)]),
                    in1=_ap(prod, HD, [(D * D, G * T), (D, D), (1, HD)]))
                vector.tensor_add(
                    out=_ap(o2b, 0, [(D, G * T), (1, D)]),
                    in0=_ap(p3, 0, [(D * HD, G * T), (HD, D)]),
                    in1=_ap(p3, 1, [(D * HD, G * T), (HD, D)]))
                vector.tensor_add(
                    out=_ap(res, s * XIN_SET, [(D, G * T), (1, D)]),
                    in0=_ap(o2b, 0, [(D, G * T), (1, D)]),
                    in1=_ap(p3, 2, [(D * HD, G * T), (HD, D)])
                ).then_inc(sems["res_done"], 1)

            for n in range(NT):
                phase_a(n)
                if n >= 1:
                    phase_b(n - 1)
            phase_b(NT - 1)

    return nc


def _pack_weights(Wq, Wk, Wv, Wo):
    from ml_dtypes import bfloat16
    wts = np.zeros(CLEN, dtype=np.float32)
    scale = 1.0 / math.sqrt(HD)
    A2 = wts[OFF_A2:OFF_A2 + 36].reshape(2, D, POS)
    for h in range(NH):
        A2[0, h * HD:(h + 1) * HD, :] = (Wq[h * HD:(h + 1) * HD, :].T
                                         @ Wk[h * HD:(h + 1) * HD, :]) * scale
        A2[1, h * HD:(h + 1) * HD, :] = Wv[h * HD:(h + 1) * HD, :]
    wtsb = np.zeros(CBLEN, dtype=np.float32)
    mask = np.where(np.tril(np.ones((T, T))) > 0, 0.0, -1e9).astype(np.float32)
    wtsb[OFF_MASK:OFF_MASK + TT] = mask.reshape(-1)
    wtsb[OFF_WO:OFF_WO + 36] = Wo.reshape(-1)
    return wts, wtsb.astype(bfloat16)


@lru_cache(maxsize=2)
def _cached_kernel(bc, G):
    return build_kernel(bc, G)


def kernel(x, Wq, Wk, Wv, Wo):
    x = np.ascontiguousarray(x, dtype=np.float32)
    B = x.shape[0]
    bc = B // NCORES
    G = 4
    nc = _cached_kernel(bc, G)
    wts, wtsb = _pack_weights(np.asarray(Wq, dtype=np.float32),
                              np.asarray(Wk, dtype=np.float32),
                              np.asarray(Wv, dtype=np.float32),
                              np.asarray(Wo, dtype=np.float32))
    in_maps = [{"x": x[i * bc:(i + 1) * bc], "wts": wts, "wtsb": wtsb}
               for i in range(NCORES)]
    r = run_bass_kernel_spmd(nc, in_maps, core_ids=list(range(NCORES)))
    return np.concatenate([m["out"] for m in r.results], axis=0)


# revision 28
# speedup vs baseline: 1.1449x; 1.0167x over previous
"""Trainium2 Bass kernel for nn_Attn_40046275068166.

Tiny causal MHA over huge batch: x[B=65536, T=34, D=6], 2 heads, head_dim 3.
Pure data parallelism over 8 cores (batch sharded), batch on the 128 SBUF
partitions inside each core, G=4 examples per partition per tile.

v3 design:
- All score/exp/PV/output-projection elementwise work in bf16; DVE
  tensor_tensor ops with every operand 2-byte + inner-stride-1 run in the
  2x perf mode. Reduces keep fp32 outputs (accuracy) and run 1x.
- The score outer-product's broadcast operand xp[i,a] (stride-0 along j,
  which disqualifies 2x) is materialized into packed bf16 planes xbc by
  the otherwise-idle ACT engine (Copy activation), making the score muls
  2x-eligible.
- exp runs on ACT in-place over the causal blocks only; dead quarter of
  the score plane is never written or read (no memsets).
- Projections (y = A_h xp, v = Wv xt; fp32 accumulate, bf16 store) run on
  the GPSIMD engine, overlapped with DVE work.
- Engines per tile n: GPSIMD proj(n)+DMA, ACT xbc(n)+exp(n-1),
  DVE phase_a(n) [scores] + phase_b(n-1) [softmax+PV+outproj].

Math identity: s[b,h,i,j] = xp_i^T A_h xp_j with A_h = Wq_h^T Wk_h/sqrt(hd),
so only y = A_h xp and v = Wv xt are projected and s = xp_i . y_j.
Causal mask applied additively (-1e9, bf16) on the two diagonal blocks.

Raw bass with explicit semaphores - this walrus build allows at most one
sync-wait per instruction, so multi-dependencies are standalone wait ops.
"""

import math
from contextlib import ExitStack
from functools import lru_cache

import numpy as np

import concourse.bass as bass
from concourse import mybir
from concourse.bass_utils import run_bass_kernel_spmd

NCORES = 8
T = 34
D = 6
NH = 2
HD = 3
POS = 3
TT = T * T          # 1156
STT = NH * TT
P = 128

F32 = mybir.dt.float32
BF16 = mybir.dt.bfloat16

# fp32 constants (projection weights)
OFF_A2 = 0          # [2][6][3]  w=0: y-proj weights, w=1: v-proj weights
CLEN = 36
# bf16 constants
OFF_MASK = 0        # [1156]     additive causal mask (0 / -1e9)
OFF_WO = TT         # [6][6]     WoM[dm][e]
CBLEN = TT + 36


def _ap(t, off, dims):
    p0 = t[:].ap[0]
    return bass.AP(tensor=t, offset=off, ap=[list(p0)] + [list(d) for d in dims])


def build_kernel(bc, G):
    assert bc % (P * G) == 0
    NT = bc // (P * G)

    nc = bass.Bass("TRN2")
    x = nc.dram_tensor("x", [bc, T, D], F32, kind="ExternalInput")
    wts = nc.dram_tensor("wts", [CLEN], F32, kind="ExternalInput")
    wtsb = nc.dram_tensor("wtsb", [CBLEN], BF16, kind="ExternalInput")
    out = nc.dram_tensor("out", [bc, T, D], F32, kind="ExternalOutput")

    xr = x[:].rearrange("(n g p) t d -> n p g t d", g=G, p=P)
    outr = out[:].rearrange("(n g p) t d -> n p g t d", g=G, p=P)
    wts_b = bass.AP(tensor=wts, offset=0, ap=[[0, P], [1, CLEN]])
    wtsb_b = bass.AP(tensor=wtsb, offset=0, ap=[[0, P], [1, CBLEN]])

    with ExitStack() as ctx:
        sb = lambda nm, shape, dt=F32: ctx.enter_context(
            nc.sbuf_tensor(nm, shape, dt))
        wsb = sb("wsb", [P, CLEN])
        wsbb = sb("wsbb", [P, CBLEN], BF16)
        xin = sb("xin", [P, 2, G, T, D])
        yv = sb("yv", [P, 3, 2, G, D, T], BF16)   # [n%3][w][g][hc][j]
        pp = sb("pp", [P, G, NH, T, T], BF16)
        t1 = sb("t1", [P, 2, G, NH, T, T], BF16)
        scr = sb("scr", [P, G, NH, T, T], BF16)   # vector-private
        xbc = sb("xbc", [P, POS, G, T, T], BF16)  # ACT-written xp planes
        den = sb("den", [P, G, NH, T])
        rcp = sb("rcp", [P, G, NH, T])
        o2 = sb("o2", [P, G, T, D])
        o2b = sb("o2b", [P, G, T, D], BF16)
        prod = sb("prod", [P, G, T, D, D], BF16)  # [(g,t)][dm][e]
        p3 = sb("p3", [P, G, T, D, HD], BF16)
        res = sb("res", [P, 2, G, T, D])
        pacc = sb("pacc", [P, G, D, T])           # gpsimd-private
        ptmp = sb("ptmp", [P, G, D, T])
        # bf16 fold-tree scratches (vector-private, reused per reduction)
        f17 = sb("f17", [P, G, NH, 17, 17], BF16)
        tw8 = sb("tw8", [P, G, NH, T, 8], BF16)
        tw4 = sb("tw4", [P, G, NH, T, 4], BF16)
        tw2 = sb("tw2", [P, G, NH, T, 2], BF16)
        tsum = sb("tsum", [P, G, NH, T], BF16)

        sem_names = ["dma_in0", "dma_in1", "const", "constb", "proj_done",
                     "a_done", "s_done", "e_done", "b_done", "res_done",
                     "d_done", "out0", "out1"]
        sems = {k: ctx.enter_context(nc.semaphore(name=k)) for k in sem_names}

        XIN_SET = G * T * D
        XIN_G = T * D
        YV_BUF = 2 * G * T * D
        YV_W = G * T * D
        YV_G = T * D
        TS_SET = G * NH * TT
        H = T // 2
        BLKS = [(0, 0), (H, 0), (H, H)]

        block = ctx.enter_context(nc.Block())

        @block.gpsimd
        def _(sync):
            def store(k):
                sp = k % 2
                sync.wait_ge(sems["res_done"], k + 1)
                sync.dma_start(
                    out=outr[k],
                    in_=_ap(res, sp * XIN_SET, [(XIN_G, G), (1, T * D)]),
                ).then_inc(sems["out0" if sp == 0 else "out1"], 16)

            def load(n):
                s = n % 2
                if n >= 2:
                    # xin[s] free once ACT's xbc(n-2) has read it
                    sync.wait_ge(sems["a_done"], n - 1)
                sync.dma_start(
                    out=_ap(xin, s * XIN_SET, [(XIN_G, G), (1, T * D)]),
                    in_=xr[n],
                ).then_inc(sems["dma_in0" if s == 0 else "dma_in1"], 16)

            def proj(n):
                s = n % 2
                m = n % 3
                sync.wait_ge(sems["dma_in0" if s == 0 else "dma_in1"],
                             16 * (n // 2 + 1))
                if n >= 3:
                    # WAR: yv[m] last read by PV muls of phase_b(n-3)
                    sync.wait_ge(sems["b_done"], n - 2)
                last = None
                for w in range(2):
                    xoff = s * XIN_SET + (3 - 3 * w)
                    for b in range(POS):
                        i0 = _ap(xin, xoff + b, [(XIN_G, G), (0, D), (D, T)])
                        i1 = _ap(wsb, OFF_A2 + w * 18 + b,
                                 [(0, G), (3, D), (0, T)])
                        if b == 0:
                            sync.tensor_mul(
                                out=_ap(pacc, 0, [(D * T, G), (T, D), (1, T)]),
                                in0=i0, in1=i1)
                        else:
                            sync.tensor_mul(
                                out=_ap(ptmp, 0, [(D * T, G), (T, D), (1, T)]),
                                in0=i0, in1=i1)
                            dst = (_ap(yv, m * YV_BUF + w * YV_W,
                                       [(YV_G, G), (1, T * D)]) if b == 2
                                   else _ap(pacc, 0, [(D * T, G), (1, T * D)]))
                            last = sync.tensor_add(
                                out=dst,
                                in0=_ap(pacc, 0, [(D * T, G), (1, T * D)]),
                                in1=_ap(ptmp, 0, [(D * T, G), (1, T * D)]))
                last.then_inc(sems["proj_done"], 1)

            sync.dma_start(out=wsb[:], in_=wts_b).then_inc(sems["const"], 16)
            sync.dma_start(out=wsbb[:], in_=wtsb_b).then_inc(
                sems["constb"], 16)
            load(0)
            load(1)
            sync.wait_ge(sems["const"], 16)
            for n in range(NT):
                if n == 0:
                    # tile 0's projection runs on vector (startup warmup);
                    # xin[0] also read there, so gate its reuse
                    sync.wait_ge(sems["d_done"], 1)
                else:
                    proj(n)
                if n >= 2:
                    store(n - 2)
                if n + 2 < NT:
                    load(n + 2)
            store(NT - 2)
            store(NT - 1)
            sync.wait_ge(sems["out0"], 16 * ((NT + 1) // 2))
            sync.wait_ge(sems["out1"], 16 * (NT // 2))

        @block.scalar
        def _(scalar):
            def xbc_fill(n):
                # xbc[a, g, i, j] = xp[g, i, a]  (bf16, j-packed), causal
                # blocks only: AB column (all rows, j<H) + C diag block.
                s = n % 2
                scalar.wait_ge(sems["dma_in0" if s == 0 else "dma_in1"],
                               16 * (n // 2 + 1))
                last = None
                for a in range(POS):
                    for (ro, io, li, lj) in ((0, 0, T, H),
                                             (H * T + H, H * D, T - H, T - H)):
                        last = scalar.activation(
                            out=_ap(xbc, a * G * TT + ro,
                                    [(TT, G), (T, li), (1, lj)]),
                            in_=_ap(xin, s * XIN_SET + 3 + a + io,
                                    [(XIN_G, G), (D, li), (0, lj)]),
                            func=mybir.ActivationFunctionType.Copy,
                        )
                last.then_inc(sems["a_done"], 1)

            xbc_fill(0)
            for n in range(NT):
                s = n % 2
                if n >= 2:
                    scalar.wait_ge(sems["b_done"], n - 1)
                scalar.wait_ge(sems["s_done"], n + 1)
                # exp in-place over causal blocks of t1[s]
                scalar.activation(
                    out=_ap(t1, s * TS_SET, [(TT, G * NH), (T, T), (1, H)]),
                    in_=_ap(t1, s * TS_SET, [(TT, G * NH), (T, T), (1, H)]),
                    func=mybir.ActivationFunctionType.Exp,
                )
                ro = H * T + H
                scalar.activation(
                    out=_ap(t1, s * TS_SET + ro,
                            [(TT, G * NH), (T, T - H), (1, T - H)]),
                    in_=_ap(t1, s * TS_SET + ro,
                            [(TT, G * NH), (T, T - H), (1, T - H)]),
                    func=mybir.ActivationFunctionType.Exp,
                ).then_inc(sems["e_done"], 1)
                if n + 1 < NT:
                    xbc_fill(n + 1)

        @block.vector
        def _(vector):
            vector.wait_ge(sems["constb"], 16)
            vector.wait_ge(sems["const"], 16)
            vector.wait_ge(sems["dma_in0"], 16)
            # tile-0 projection on vector (gpsimd's first proj would stall
            # the pipeline ~22us); fp32 scratch borrowed from res[1]/o2,
            # both first written much later
            last = None
            for w in range(2):
                xoff = 3 - 3 * w
                for b in range(POS):
                    i0 = _ap(xin, xoff + b, [(XIN_G, G), (0, D), (D, T)])
                    i1 = _ap(wsb, OFF_A2 + w * 18 + b,
                             [(0, G), (3, D), (0, T)])
                    if b == 0:
                        vector.tensor_mul(
                            out=_ap(res, XIN_SET,
                                    [(D * T, G), (T, D), (1, T)]),
                            in0=i0, in1=i1)
                    else:
                        vector.tensor_mul(
                            out=_ap(o2, 0, [(D * T, G), (T, D), (1, T)]),
                            in0=i0, in1=i1)
                        dst = (_ap(yv, w * YV_W, [(YV_G, G), (1, T * D)])
                               if b == 2
                               else _ap(res, XIN_SET,
                                        [(D * T, G), (1, T * D)]))
                        last = vector.tensor_add(
                            out=dst,
                            in0=_ap(res, XIN_SET, [(D * T, G), (1, T * D)]),
                            in1=_ap(o2, 0, [(D * T, G), (1, T * D)]))
            last.then_inc(sems["d_done"], 1)

            def phase_a(n):
                s = n % 2
                m = n % 3
                if n >= 1:
                    vector.wait_ge(sems["proj_done"], n)
                vector.wait_ge(sems["a_done"], n + 1)
                # scores: t1[g,h,i,j] = sum_a xbc[a,g,i,j] * y[g,(h,a),j]
                # two merged causal regions: AB column (all rows, j<H) and
                # C diag block
                for a in range(POS):
                    dst = t1 if a == 0 else scr
                    doff = s * TS_SET if a == 0 else 0
                    for h in range(NH):
                        for (ro, li) in ((0, T), (H * T + H, T - H)):
                            vector.tensor_mul(
                                out=_ap(dst, doff + h * TT + ro,
                                        [(NH * TT, G), (T, li), (1, H)]),
                                in0=_ap(xbc, a * G * TT + ro,
                                        [(TT, G), (T, li), (1, H)]),
                                in1=_ap(yv, m * YV_BUF + (h * HD + a) * T
                                        + (0 if ro == 0 else H),
                                        [(YV_G, G), (0, li), (1, H)]))
                    if a == 2:
                        # scr += mask on diagonal blocks A and C
                        for ro in (0, H * T + H):
                            vector.tensor_add(
                                out=_ap(scr, ro, [(TT, G * NH), (T, H), (1, H)]),
                                in0=_ap(scr, ro, [(TT, G * NH), (T, H), (1, H)]),
                                in1=_ap(wsbb, OFF_MASK + ro,
                                        [(0, G * NH), (T, H), (1, H)]))
                    if a > 0:
                        # t1 += scr over AB column + C block
                        mm = None
                        for (ro, li, lj) in ((0, T, H), (H * T + H, T - H, T - H)):
                            mm = vector.tensor_add(
                                out=_ap(t1, s * TS_SET + ro,
                                        [(TT, G * NH), (T, li), (1, lj)]),
                                in0=_ap(t1, s * TS_SET + ro,
                                        [(TT, G * NH), (T, li), (1, lj)]),
                                in1=_ap(scr, ro,
                                        [(TT, G * NH), (T, li), (1, lj)]))
                        if a == 2:
                            mm.then_inc(sems["s_done"], 1)

            GH = G * NH

            def fold_tree(src, soff):
                """bf16 pairwise row-sum tree over causal planes of src.

                After this: tsum[g,h,i] + leftover col 16 (A rows: src[i,16],
                BC rows: f17[i-17,16]) equals the causal row sum."""
                # L1: rows 17-33: fold j and j+17
                vector.tensor_add(
                    out=_ap(f17, 0, [(289, GH), (17, 17), (1, 17)]),
                    in0=_ap(src, soff + H * T, [(TT, GH), (T, 17), (1, 17)]),
                    in1=_ap(src, soff + H * T + H,
                            [(TT, GH), (T, 17), (1, 17)]))
                # L2a: A rows (0-16): j pairs (0-7)+(8-15)
                vector.tensor_add(
                    out=_ap(tw8, 0, [(T * 8, GH), (8, 17), (1, 8)]),
                    in0=_ap(src, soff, [(TT, GH), (T, 17), (1, 8)]),
                    in1=_ap(src, soff + 8, [(TT, GH), (T, 17), (1, 8)]))
                # L2f: folded rows -> tw8 rows 17-33
                vector.tensor_add(
                    out=_ap(tw8, 17 * 8, [(T * 8, GH), (8, 17), (1, 8)]),
                    in0=_ap(f17, 0, [(289, GH), (17, 17), (1, 8)]),
                    in1=_ap(f17, 8, [(289, GH), (17, 17), (1, 8)]))
                # L3: 8->4, L4: 4->2
                vector.tensor_add(
                    out=_ap(tw4, 0, [(T * 4, GH), (4, T), (1, 4)]),
                    in0=_ap(tw8, 0, [(T * 8, GH), (8, T), (1, 4)]),
                    in1=_ap(tw8, 4, [(T * 8, GH), (8, T), (1, 4)]))
                vector.tensor_add(
                    out=_ap(tw2, 0, [(T * 2, GH), (2, T), (1, 2)]),
                    in0=_ap(tw4, 0, [(T * 4, GH), (4, T), (1, 2)]),
                    in1=_ap(tw4, 2, [(T * 4, GH), (4, T), (1, 2)]))
                # L5: 2->1
                vector.tensor_add(
                    out=_ap(tsum, 0, [(T, GH), (1, T)]),
                    in0=_ap(tw2, 0, [(T * 2, GH), (2, T)]),
                    in1=_ap(tw2, 1, [(T * 2, GH), (2, T)]))

            def phase_b(n):
                s = n % 2
                m = n % 3
                vector.wait_ge(sems["e_done"], n + 1)
                if n >= 2:
                    vector.wait_ge(sems["out0" if s == 0 else "out1"],
                                   16 * (n // 2))
                # den: fold tree + leftover col, fp32 out
                fold_tree(t1, s * TS_SET)
                vector.tensor_add(
                    out=_ap(den, 0, [(T, GH), (1, 17)]),
                    in0=_ap(tsum, 0, [(T, GH), (1, 17)]),
                    in1=_ap(t1, s * TS_SET + 16, [(TT, GH), (T, 17)]))
                vector.tensor_add(
                    out=_ap(den, H, [(T, GH), (1, 17)]),
                    in0=_ap(tsum, 17, [(T, GH), (1, 17)]),
                    in1=_ap(f17, 16, [(289, GH), (17, 17)]))
                vector.reciprocal(
                    out=_ap(rcp, 0, [(1, G * NH * T)]),
                    in_=_ap(den, 0, [(1, G * NH * T)]))
                for c in range(HD):
                    for h in range(NH):
                        for bi, (ro, li) in enumerate(((0, T),
                                                      (H * T + H, T - H))):
                            mm = vector.tensor_mul(
                                out=_ap(pp, h * TT + ro,
                                        [(NH * TT, G), (T, li), (1, H)]),
                                in0=_ap(t1, s * TS_SET + h * TT + ro,
                                        [(NH * TT, G), (T, li), (1, H)]),
                                in1=_ap(yv, m * YV_BUF + YV_W + (h * HD + c) * T
                                        + (0 if ro == 0 else H),
                                        [(YV_G, G), (0, li), (1, H)]))
                            if c == HD - 1 and h == NH - 1 and bi == 1:
                                mm.then_inc(sems["b_done"], 1)
                    fold_tree(pp, 0)
                    vector.tensor_add(
                        out=_ap(o2, c, [(T * D, G), (HD, NH), (D, 17)]),
                        in0=_ap(tsum, 0, [(NH * T, G), (T, NH), (1, 17)]),
                        in1=_ap(pp, 16, [(NH * TT, G), (TT, NH), (T, 17)]))
                    vector.tensor_add(
                        out=_ap(o2, c + H * D, [(T * D, G), (HD, NH), (D, 17)]),
                        in0=_ap(tsum, 17, [(NH * T, G), (T, NH), (1, 17)]),
                        in1=_ap(f17, 16, [(NH * 289, G), (289, NH), (17, 17)]))
                for g in range(G):
                    vector.tensor_mul(
                        out=_ap(o2b, g * T * D, [(D, T), (HD, NH), (1, HD)]),
                        in0=_ap(o2, g * T * D, [(D, T), (HD, NH), (1, HD)]),
                        in1=_ap(rcp, g * NH * T, [(1, T), (T, NH), (0, HD)]))
                # outproj: prod[(g,t),dm,e] = o2b[g,t,e] * Wo[dm,e], fold
                # e-halves, reduce remaining 3
                vector.tensor_mul(
                    out=_ap(prod, 0, [(D * D, G * T), (D, D), (1, D)]),
                    in0=_ap(o2b, 0, [(D, G * T), (0, D), (1, D)]),
                    in1=_ap(wsbb, OFF_WO, [(0, G * T), (D, D), (1, D)]))
                vector.tensor_add(
                    out=_ap(p3, 0, [(D * HD, G * T), (HD, D), (1, HD)]),
                    in0=_ap(prod, 0, [(D * D, G * T), (D, D), (1, HD)]),
                    in1=_ap(prod, HD, [(D * D, G * T), (D, D), (1, HD)]))
                vector.tensor_add(
                    out=_ap(o2b, 0, [(D, G * T), (1, D)]),
                    in0=_ap(p3, 0, [(D * HD, G * T), (HD, D)]),
                    in1=_ap(p3, 1, [(D * HD, G * T), (HD, D)]))
                vector.tensor_add(
                    out=_ap(res, s * XIN_SET, [(D, G * T), (1, D)]),
                    in0=_ap(o2b, 0, [(D, G * T), (1, D)]),
                    in1=_ap(p3, 2, [(D * HD, G * T), (HD, D)])
                ).then_inc(sems["res_done"], 1)

            for n in range(NT):
                phase_a(n)
                if n >= 1:
                    phase_b(n - 1)
            phase_b(NT - 1)

    return nc


def _pack_weights(Wq, Wk, Wv, Wo):
    from ml_dtypes import bfloat16
    wts = np.zeros(CLEN, dtype=np.float32)
    scale = 1.0 / math.sqrt(HD)
    A2 = wts[OFF_A2:OFF_A2 + 36].reshape(2, D, POS)
    for h in range(NH):
        A2[0, h * HD:(h + 1) * HD, :] = (Wq[h * HD:(h + 1) * HD, :].T
                                         @ Wk[h * HD:(h + 1) * HD, :]) * scale
        A2[1, h * HD:(h + 1) * HD, :] = Wv[h * HD:(h + 1) * HD, :]
    wtsb = np.zeros(CBLEN, dtype=np.float32)
    mask = np.where(np.tril(np.ones((T, T))) > 0, 0.0, -1e9).astype(np.float32)
    wtsb[OFF_MASK:OFF_MASK + TT] = mask.reshape(-1)
    wtsb[OFF_WO:OFF_WO + 36] = Wo.reshape(-1)
    return wts, wtsb.astype(bfloat16)


@lru_cache(maxsize=2)
def _cached_kernel(bc, G):
    return build_kernel(bc, G)


def kernel(x, Wq, Wk, Wv, Wo):
    x = np.ascontiguousarray(x, dtype=np.float32)
    B = x.shape[0]
    bc = B // NCORES
    G = 4
    nc = _cached_kernel(bc, G)
    wts, wtsb = _pack_weights(np.asarray(Wq, dtype=np.float32),
                              np.asarray(Wk, dtype=np.float32),
                              np.asarray(Wv, dtype=np.float32),
                              np.asarray(Wo, dtype=np.float32))
    in_maps = [{"x": x[i * bc:(i + 1) * bc], "wts": wts, "wtsb": wtsb}
               for i in range(NCORES)]
    r = run_bass_kernel_spmd(nc, in_maps, core_ids=list(range(NCORES)))
    return np.concatenate([m["out"] for m in r.results], axis=0)


# revision 30
# speedup vs baseline: 1.1463x; 1.0013x over previous
"""Trainium2 Bass kernel for nn_Attn_40046275068166.

Tiny causal MHA over huge batch: x[B=65536, T=34, D=6], 2 heads, head_dim 3.
Pure data parallelism over 8 cores (batch sharded), batch on the 128 SBUF
partitions inside each core, G=4 examples per partition per tile.

v3 design:
- All score/exp/PV/output-projection elementwise work in bf16; DVE
  tensor_tensor ops with every operand 2-byte + inner-stride-1 run in the
  2x perf mode. Reduces keep fp32 outputs (accuracy) and run 1x.
- The score outer-product's broadcast operand xp[i,a] (stride-0 along j,
  which disqualifies 2x) is materialized into packed bf16 planes xbc by
  the otherwise-idle ACT engine (Copy activation), making the score muls
  2x-eligible.
- exp runs on ACT in-place over the causal blocks only; dead quarter of
  the score plane is never written or read (no memsets).
- Projections (y = A_h xp, v = Wv xt; fp32 accumulate, bf16 store) run on
  the GPSIMD engine, overlapped with DVE work.
- Engines per tile n: GPSIMD proj(n)+DMA, ACT xbc(n)+exp(n-1),
  DVE phase_a(n) [scores] + phase_b(n-1) [softmax+PV+outproj].

Math identity: s[b,h,i,j] = xp_i^T A_h xp_j with A_h = Wq_h^T Wk_h/sqrt(hd),
so only y = A_h xp and v = Wv xt are projected and s = xp_i . y_j.
Causal mask applied additively (-1e9, bf16) on the two diagonal blocks.

Raw bass with explicit semaphores - this walrus build allows at most one
sync-wait per instruction, so multi-dependencies are standalone wait ops.
"""

import math
from contextlib import ExitStack
from functools import lru_cache

import numpy as np

import concourse.bass as bass
from concourse import mybir
from concourse.bass_utils import run_bass_kernel_spmd

NCORES = 8
T = 34
D = 6
NH = 2
HD = 3
POS = 3
TT = T * T          # 1156
STT = NH * TT
P = 128

F32 = mybir.dt.float32
BF16 = mybir.dt.bfloat16

# fp32 constants (projection weights)
OFF_A2 = 0          # [2][6][3]  w=0: y-proj weights, w=1: v-proj weights
CLEN = 36
# bf16 constants
OFF_MASK = 0        # [1156]     additive causal mask (0 / -1e9)
OFF_WO = TT         # [6][6]     WoM[dm][e]
CBLEN = TT + 36


def _ap(t, off, dims):
    p0 = t[:].ap[0]
    return bass.AP(tensor=t, offset=off, ap=[list(p0)] + [list(d) for d in dims])


def build_kernel(bc, G):
    assert bc % (P * G) == 0
    NT = bc // (P * G)

    nc = bass.Bass("TRN2")
    x = nc.dram_tensor("x", [bc, T, D], F32, kind="ExternalInput")
    wts = nc.dram_tensor("wts", [CLEN], F32, kind="ExternalInput")
    wtsb = nc.dram_tensor("wtsb", [CBLEN], BF16, kind="ExternalInput")
    out = nc.dram_tensor("out", [bc, T, D], F32, kind="ExternalOutput")

    xr = x[:].rearrange("(n g p) t d -> n p g t d", g=G, p=P)
    outr = out[:].rearrange("(n g p) t d -> n p g t d", g=G, p=P)
    wts_b = bass.AP(tensor=wts, offset=0, ap=[[0, P], [1, CLEN]])
    wtsb_b = bass.AP(tensor=wtsb, offset=0, ap=[[0, P], [1, CBLEN]])

    with ExitStack() as ctx:
        sb = lambda nm, shape, dt=F32: ctx.enter_context(
            nc.sbuf_tensor(nm, shape, dt))
        wsb = sb("wsb", [P, CLEN])
        wsbb = sb("wsbb", [P, CBLEN], BF16)
        xin = sb("xin", [P, 2, G, T, D])
        yv = sb("yv", [P, 3, 2, G, D, T], BF16)   # [n%3][w][g][hc][j]
        pp = sb("pp", [P, G, NH, T, T], BF16)
        t1 = sb("t1", [P, 2, G, NH, T, T], BF16)
        scr = sb("scr", [P, G, NH, T, T], BF16)   # vector-private
        xbc = sb("xbc", [P, POS, G, T, T], BF16)  # ACT-written xp planes
        den = sb("den", [P, G, NH, T])
        rcp = sb("rcp", [P, G, NH, T])
        o2 = sb("o2", [P, G, T, D])
        o2b = sb("o2b", [P, G, T, D], BF16)
        prod = sb("prod", [P, G, T, D, D], BF16)  # [(g,t)][dm][e]
        p3 = sb("p3", [P, G, T, D, HD], BF16)
        res = sb("res", [P, 2, G, T, D])
        pacc = sb("pacc", [P, G, D, T])           # gpsimd-private
        ptmp = sb("ptmp", [P, G, D, T])
        # bf16 fold-tree scratches (vector-private, reused per reduction)
        f17 = sb("f17", [P, G, NH, 17, 17], BF16)
        tw8 = sb("tw8", [P, G, NH, T, 8], BF16)
        tw4 = sb("tw4", [P, G, NH, T, 4], BF16)
        tw2 = sb("tw2", [P, G, NH, T, 2], BF16)
        tsum = sb("tsum", [P, G, NH, T], BF16)

        sem_names = ["dma_in0", "dma_in1", "const", "constb", "proj_done",
                     "a_done", "s_done", "e_done", "b_done", "res_done",
                     "d_done", "out0", "out1"]
        sems = {k: ctx.enter_context(nc.semaphore(name=k)) for k in sem_names}

        XIN_SET = G * T * D
        XIN_G = T * D
        YV_BUF = 2 * G * T * D
        YV_W = G * T * D
        YV_G = T * D
        TS_SET = G * NH * TT
        H = T // 2
        BLKS = [(0, 0), (H, 0), (H, H)]

        block = ctx.enter_context(nc.Block())

        @block.gpsimd
        def _(sync):
            def store(k):
                sp = k % 2
                sync.wait_ge(sems["res_done"], k + 1)
                sync.dma_start(
                    out=outr[k],
                    in_=_ap(res, sp * XIN_SET, [(XIN_G, G), (1, T * D)]),
                ).then_inc(sems["out0" if sp == 0 else "out1"], 16)

            def load(n):
                s = n % 2
                if n >= 2:
                    # xin[s] free once ACT's xbc(n-2) has read it
                    sync.wait_ge(sems["a_done"], n - 1)
                sync.dma_start(
                    out=_ap(xin, s * XIN_SET, [(XIN_G, G), (1, T * D)]),
                    in_=xr[n],
                ).then_inc(sems["dma_in0" if s == 0 else "dma_in1"], 16)

            def proj(n):
                s = n % 2
                m = n % 3
                sync.wait_ge(sems["dma_in0" if s == 0 else "dma_in1"],
                             16 * (n // 2 + 1))
                if n >= 3:
                    # WAR: yv[m] last read by PV muls of phase_b(n-3)
                    sync.wait_ge(sems["b_done"], n - 2)
                last = None
                for w in range(2):
                    xoff = s * XIN_SET + (3 - 3 * w)
                    for b in range(POS):
                        i0 = _ap(xin, xoff + b, [(XIN_G, G), (0, D), (D, T)])
                        i1 = _ap(wsb, OFF_A2 + w * 18 + b,
                                 [(0, G), (3, D), (0, T)])
                        if b == 0:
                            sync.tensor_mul(
                                out=_ap(pacc, 0, [(D * T, G), (T, D), (1, T)]),
                                in0=i0, in1=i1)
                        else:
                            sync.tensor_mul(
                                out=_ap(ptmp, 0, [(D * T, G), (T, D), (1, T)]),
                                in0=i0, in1=i1)
                            dst = (_ap(yv, m * YV_BUF + w * YV_W,
                                       [(YV_G, G), (1, T * D)]) if b == 2
                                   else _ap(pacc, 0, [(D * T, G), (1, T * D)]))
                            last = sync.tensor_add(
                                out=dst,
                                in0=_ap(pacc, 0, [(D * T, G), (1, T * D)]),
                                in1=_ap(ptmp, 0, [(D * T, G), (1, T * D)]))
                last.then_inc(sems["proj_done"], 1)

            sync.dma_start(out=wsb[:], in_=wts_b).then_inc(sems["const"], 16)
            sync.dma_start(out=wsbb[:], in_=wtsb_b).then_inc(
                sems["constb"], 16)
            load(0)
            load(1)
            sync.wait_ge(sems["const"], 16)
            for n in range(NT):
                if n == 0:
                    # tile 0's projection runs on vector (startup warmup);
                    # xin[0] also read there, so gate its reuse
                    sync.wait_ge(sems["d_done"], 1)
                else:
                    proj(n)
                if n >= 2:
                    store(n - 2)
                if n + 2 < NT:
                    load(n + 2)
            store(NT - 2)
            store(NT - 1)
            sync.wait_ge(sems["out0"], 16 * ((NT + 1) // 2))
            sync.wait_ge(sems["out1"], 16 * (NT // 2))

        @block.scalar
        def _(scalar):
            def xbc_fill(n):
                # xbc[a, g, i, j] = xp[g, i, a]  (bf16, j-packed), causal
                # blocks only: AB column (all rows, j<H) + C diag block.
                s = n % 2
                scalar.wait_ge(sems["dma_in0" if s == 0 else "dma_in1"],
                               16 * (n // 2 + 1))
                last = None
                for a in range(POS):
                    for (ro, io, li, lj) in ((0, 0, T, H),
                                             (H * T + H, H * D, T - H, T - H)):
                        last = scalar.activation(
                            out=_ap(xbc, a * G * TT + ro,
                                    [(TT, G), (T, li), (1, lj)]),
                            in_=_ap(xin, s * XIN_SET + 3 + a + io,
                                    [(XIN_G, G), (D, li), (0, lj)]),
                            func=mybir.ActivationFunctionType.Copy,
                        )
                last.then_inc(sems["a_done"], 1)

            xbc_fill(0)
            for n in range(NT):
                s = n % 2
                if n >= 2:
                    scalar.wait_ge(sems["b_done"], n - 1)
                scalar.wait_ge(sems["s_done"], n + 1)
                # exp in-place over causal blocks of t1[s]
                scalar.activation(
                    out=_ap(t1, s * TS_SET, [(TT, G * NH), (T, T), (1, H)]),
                    in_=_ap(t1, s * TS_SET, [(TT, G * NH), (T, T), (1, H)]),
                    func=mybir.ActivationFunctionType.Exp,
                )
                ro = H * T + H
                scalar.activation(
                    out=_ap(t1, s * TS_SET + ro,
                            [(TT, G * NH), (T, T - H), (1, T - H)]),
                    in_=_ap(t1, s * TS_SET + ro,
                            [(TT, G * NH), (T, T - H), (1, T - H)]),
                    func=mybir.ActivationFunctionType.Exp,
                ).then_inc(sems["e_done"], 1)
                if n + 1 < NT:
                    xbc_fill(n + 1)

        @block.vector
        def _(vector):
            vector.wait_ge(sems["constb"], 16)
            vector.wait_ge(sems["const"], 16)
            vector.wait_ge(sems["dma_in0"], 16)
            # tile-0 projection on vector (gpsimd's first proj would stall
            # the pipeline ~22us); fp32 scratch borrowed from res[1]/o2,
            # both first written much later
            last = None
            for w in range(2):
                xoff = 3 - 3 * w
                for b in range(POS):
                    i0 = _ap(xin, xoff + b, [(XIN_G, G), (0, D), (D, T)])
                    i1 = _ap(wsb, OFF_A2 + w * 18 + b,
                             [(0, G), (3, D), (0, T)])
                    if b == 0:
                        vector.tensor_mul(
                            out=_ap(res, XIN_SET,
                                    [(D * T, G), (T, D), (1, T)]),
                            in0=i0, in1=i1)
                    else:
                        vector.tensor_mul(
                            out=_ap(o2, 0, [(D * T, G), (T, D), (1, T)]),
                            in0=i0, in1=i1)
                        dst = (_ap(yv, w * YV_W, [(YV_G, G), (1, T * D)])
                               if b == 2
                               else _ap(res, XIN_SET,
                                        [(D * T, G), (1, T * D)]))
                        last = vector.tensor_add(
                            out=dst,
                            in0=_ap(res, XIN_SET, [(D * T, G), (1, T * D)]),
                            in1=_ap(o2, 0, [(D * T, G), (1, T * D)]))
            last.then_inc(sems["d_done"], 1)

            def phase_a(n):
                s = n % 2
                m = n % 3
                if n >= 1:
                    vector.wait_ge(sems["proj_done"], n)
                vector.wait_ge(sems["a_done"], n + 1)
                # scores: t1[g,h,i,j] = sum_a xbc[a,g,i,j] * y[g,(h,a),j]
                # two merged causal regions: AB column (all rows, j<H) and
                # C diag block
                for a in range(POS):
                    dst = t1 if a == 0 else scr
                    doff = s * TS_SET if a == 0 else 0
                    for h in range(NH):
                        for (ro, li) in ((0, T), (H * T + H, T - H)):
                            vector.tensor_mul(
                                out=_ap(dst, doff + h * TT + ro,
                                        [(NH * TT, G), (T, li), (1, H)]),
                                in0=_ap(xbc, a * G * TT + ro,
                                        [(TT, G), (T, li), (1, H)]),
                                in1=_ap(yv, m * YV_BUF + (h * HD + a) * T
                                        + (0 if ro == 0 else H),
                                        [(YV_G, G), (0, li), (1, H)]))
                    if a == 2:
                        # scr += mask on diagonal blocks A and C
                        for ro in (0, H * T + H):
                            vector.tensor_add(
                                out=_ap(scr, ro, [(TT, G * NH), (T, H), (1, H)]),
                                in0=_ap(scr, ro, [(TT, G * NH), (T, H), (1, H)]),
                                in1=_ap(wsbb, OFF_MASK + ro,
                                        [(0, G * NH), (T, H), (1, H)]))
                    if a > 0:
                        # t1 += scr over AB column + C block
                        mm = None
                        for (ro, li, lj) in ((0, T, H), (H * T + H, T - H, T - H)):
                            mm = vector.tensor_add(
                                out=_ap(t1, s * TS_SET + ro,
                                        [(TT, G * NH), (T, li), (1, lj)]),
                                in0=_ap(t1, s * TS_SET + ro,
                                        [(TT, G * NH), (T, li), (1, lj)]),
                                in1=_ap(scr, ro,
                                        [(TT, G * NH), (T, li), (1, lj)]))
                        if a == 2:
                            mm.then_inc(sems["s_done"], 1)

            GH = G * NH

            def fold_tree(src, soff):
                """bf16 pairwise row-sum tree over causal planes of src.

                After this: tsum[g,h,i] + leftover col 16 (A rows: src[i,16],
                BC rows: f17[i-17,16]) equals the causal row sum."""
                # L1: rows 17-33: fold j and j+17
                vector.tensor_add(
                    out=_ap(f17, 0, [(289, GH), (17, 17), (1, 17)]),
                    in0=_ap(src, soff + H * T, [(TT, GH), (T, 17), (1, 17)]),
                    in1=_ap(src, soff + H * T + H,
                            [(TT, GH), (T, 17), (1, 17)]))
                # L2a: A rows (0-16): j pairs (0-7)+(8-15)
                vector.tensor_add(
                    out=_ap(tw8, 0, [(T * 8, GH), (8, 17), (1, 8)]),
                    in0=_ap(src, soff, [(TT, GH), (T, 17), (1, 8)]),
                    in1=_ap(src, soff + 8, [(TT, GH), (T, 17), (1, 8)]))
                # L2f: folded rows -> tw8 rows 17-33
                vector.tensor_add(
                    out=_ap(tw8, 17 * 8, [(T * 8, GH), (8, 17), (1, 8)]),
                    in0=_ap(f17, 0, [(289, GH), (17, 17), (1, 8)]),
                    in1=_ap(f17, 8, [(289, GH), (17, 17), (1, 8)]))
                # L3: 8->4, L4: 4->2
                vector.tensor_add(
                    out=_ap(tw4, 0, [(T * 4, GH), (4, T), (1, 4)]),
                    in0=_ap(tw8, 0, [(T * 8, GH), (8, T), (1, 4)]),
                    in1=_ap(tw8, 4, [(T * 8, GH), (8, T), (1, 4)]))
                vector.tensor_add(
                    out=_ap(tw2, 0, [(T * 2, GH), (2, T), (1, 2)]),
                    in0=_ap(tw4, 0, [(T * 4, GH), (4, T), (1, 2)]),
                    in1=_ap(tw4, 2, [(T * 4, GH), (4, T), (1, 2)]))
                # L5: 2->1
                vector.tensor_add(
                    out=_ap(tsum, 0, [(T, GH), (1, T)]),
                    in0=_ap(tw2, 0, [(T * 2, GH), (2, T)]),
                    in1=_ap(tw2, 1, [(T * 2, GH), (2, T)]))

            def phase_b(n):
                s = n % 2
                m = n % 3
                vector.wait_ge(sems["e_done"], n + 1)
                if n >= 2:
                    vector.wait_ge(sems["out0" if s == 0 else "out1"],
                                   16 * (n // 2))
                # den: fold tree + leftover col, fp32 out
                fold_tree(t1, s * TS_SET)
                vector.tensor_add(
                    out=_ap(den, 0, [(T, GH), (1, 17)]),
                    in0=_ap(tsum, 0, [(T, GH), (1, 17)]),
                    in1=_ap(t1, s * TS_SET + 16, [(TT, GH), (T, 17)]))
                vector.tensor_add(
                    out=_ap(den, H, [(T, GH), (1, 17)]),
                    in0=_ap(tsum, 17, [(T, GH), (1, 17)]),
                    in1=_ap(f17, 16, [(289, GH), (17, 17)]))
                vector.reciprocal(
                    out=_ap(rcp, 0, [(1, G * NH * T)]),
                    in_=_ap(den, 0, [(1, G * NH * T)]))
                for c in range(HD):
                    for h in range(NH):
                        for bi, (ro, li) in enumerate(((0, T),
                                                      (H * T + H, T - H))):
                            mm = vector.tensor_mul(
                                out=_ap(pp, h * TT + ro,
                                        [(NH * TT, G), (T, li), (1, H)]),
                                in0=_ap(t1, s * TS_SET + h * TT + ro,
                                        [(NH * TT, G), (T, li), (1, H)]),
                                in1=_ap(yv, m * YV_BUF + YV_W + (h * HD + c) * T
                                        + (0 if ro == 0 else H),
                                        [(YV_G, G), (0, li), (1, H)]))
                            if c == HD - 1 and h == NH - 1 and bi == 1:
                                mm.then_inc(sems["b_done"], 1)
                    fold_tree(pp, 0)
                    vector.tensor_add(
                        out=_ap(o2, c, [(T * D, G), (HD, NH), (D, 17)]),
                        in0=_ap(tsum, 0, [(NH * T, G), (T, NH), (1, 17)]),
                        in1=_ap(pp, 16, [(NH * TT, G), (TT, NH), (T, 17)]))
                    vector.tensor_add(
                        out=_ap(o2, c + H * D, [(T * D, G), (HD, NH), (D, 17)]),
                        in0=_ap(tsum, 17, [(NH * T, G), (T, NH), (1, 17)]),
                        in1=_ap(f17, 16, [(NH * 289, G), (289, NH), (17, 17)]))
                for g in range(G):
                    vector.tensor_mul(
                        out=_ap(o2b, g * T * D, [(D, T), (HD, NH), (1, HD)]),
                        in0=_ap(o2, g * T * D, [(D, T), (HD, NH), (1, HD)]),
                        in1=_ap(rcp, g * NH * T, [(1, T), (T, NH), (0, HD)]))
                # outproj: prod[(g,t),dm,e] = o2b[g,t,e] * Wo[dm,e], fold
                # e-halves, reduce remaining 3
                vector.tensor_mul(
                    out=_ap(prod, 0, [(D * D, G * T), (D, D), (1, D)]),
                    in0=_ap(o2b, 0, [(D, G * T), (0, D), (1, D)]),
                    in1=_ap(wsbb, OFF_WO, [(0, G * T), (D, D), (1, D)]))
                vector.tensor_add(
                    out=_ap(p3, 0, [(D * HD, G * T), (HD, D), (1, HD)]),
                    in0=_ap(prod, 0, [(D * D, G * T), (D, D), (1, HD)]),
                    in1=_ap(prod, HD, [(D * D, G * T), (D, D), (1, HD)]))
                vector.tensor_add(
                    out=_ap(o2b, 0, [(D, G * T), (1, D)]),
                    in0=_ap(p3, 0, [(D * HD, G * T), (HD, D)]),
                    in1=_ap(p3, 1, [(D * HD, G * T), (HD, D)]))
                vector.tensor_add(
                    out=_ap(res, s * XIN_SET, [(D, G * T), (1, D)]),
                    in0=_ap(o2b, 0, [(D, G * T), (1, D)]),
                    in1=_ap(p3, 2, [(D * HD, G * T), (HD, D)])
                ).then_inc(sems["res_done"], 1)

            for n in range(NT):
                phase_a(n)
                if n >= 1:
                    phase_b(n - 1)
            phase_b(NT - 1)

    return nc


def _pack_weights(Wq, Wk, Wv, Wo):
    from ml_dtypes import bfloat16
    wts = np.zeros(CLEN, dtype=np.float32)
    scale = 1.0 / math.sqrt(HD)
    A2 = wts[OFF_A2:OFF_A2 + 36].reshape(2, D, POS)
    for h in range(NH):
        A2[0, h * HD:(h + 1) * HD, :] = (Wq[h * HD:(h + 1) * HD, :].T
                                         @ Wk[h * HD:(h + 1) * HD, :]) * scale
        A2[1, h * HD:(h + 1) * HD, :] = Wv[h * HD:(h + 1) * HD, :]
    wtsb = np.zeros(CBLEN, dtype=np.float32)
    mask = np.where(np.tril(np.ones((T, T))) > 0, 0.0, -1e9).astype(np.float32)
    wtsb[OFF_MASK:OFF_MASK + TT] = mask.reshape(-1)
    wtsb[OFF_WO:OFF_WO + 36] = Wo.reshape(-1)
    return wts, wtsb.astype(bfloat16)


@lru_cache(maxsize=2)
def _cached_kernel(bc, G):
    return build_kernel(bc, G)


def kernel(x, Wq, Wk, Wv, Wo):
    x = np.ascontiguousarray(x, dtype=np.float32)
    B = x.shape[0]
    bc = B // NCORES
    G = 4
    nc = _cached_kernel(bc, G)
    wts, wtsb = _pack_weights(np.asarray(Wq, dtype=np.float32),
                              np.asarray(Wk, dtype=np.float32),
                              np.asarray(Wv, dtype=np.float32),
                              np.asarray(Wo, dtype=np.float32))
    in_maps = [{"x": x[i * bc:(i + 1) * bc], "wts": wts, "wtsb": wtsb}
               for i in range(NCORES)]
    r = run_bass_kernel_spmd(nc, in_maps, core_ids=list(range(NCORES)))
    return np.concatenate([m["out"] for m in r.results], axis=0)


# revision 31
# speedup vs baseline: 1.1481x; 1.0015x over previous
"""Trainium2 Bass kernel for nn_Attn_40046275068166.

Tiny causal MHA over huge batch: x[B=65536, T=34, D=6], 2 heads, head_dim 3.
Pure data parallelism over 8 cores (batch sharded), batch on the 128 SBUF
partitions inside each core, G=4 examples per partition per tile.

v3 design:
- All score/exp/PV/output-projection elementwise work in bf16; DVE
  tensor_tensor ops with every operand 2-byte + inner-stride-1 run in the
  2x perf mode. Reduces keep fp32 outputs (accuracy) and run 1x.
- The score outer-product's broadcast operand xp[i,a] (stride-0 along j,
  which disqualifies 2x) is materialized into packed bf16 planes xbc by
  the otherwise-idle ACT engine (Copy activation), making the score muls
  2x-eligible.
- exp runs on ACT in-place over the causal blocks only; dead quarter of
  the score plane is never written or read (no memsets).
- Projections (y = A_h xp, v = Wv xt; fp32 accumulate, bf16 store) run on
  the GPSIMD engine, overlapped with DVE work.
- Engines per tile n: GPSIMD proj(n)+DMA, ACT xbc(n)+exp(n-1),
  DVE phase_a(n) [scores] + phase_b(n-1) [softmax+PV+outproj].

Math identity: s[b,h,i,j] = xp_i^T A_h xp_j with A_h = Wq_h^T Wk_h/sqrt(hd),
so only y = A_h xp and v = Wv xt are projected and s = xp_i . y_j.
Causal mask applied additively (-1e9, bf16) on the two diagonal blocks.

Raw bass with explicit semaphores - this walrus build allows at most one
sync-wait per instruction, so multi-dependencies are standalone wait ops.
"""

import math
from contextlib import ExitStack
from functools import lru_cache

import numpy as np

import concourse.bass as bass
from concourse import mybir
from concourse.bass_utils import run_bass_kernel_spmd

NCORES = 8
T = 34
D = 6
NH = 2
HD = 3
POS = 3
TT = T * T          # 1156
STT = NH * TT
P = 128

F32 = mybir.dt.float32
BF16 = mybir.dt.bfloat16

# fp32 constants (projection weights)
OFF_A2 = 0          # [2][6][3]  w=0: y-proj weights, w=1: v-proj weights
CLEN = 36
# bf16 constants
OFF_MASK = 0        # [1156]     additive causal mask (0 / -1e9)
OFF_WO = TT         # [6][6]     WoM[dm][e]
CBLEN = TT + 36


def _ap(t, off, dims):
    p0 = t[:].ap[0]
    return bass.AP(tensor=t, offset=off, ap=[list(p0)] + [list(d) for d in dims])


def build_kernel(bc, G):
    assert bc % (P * G) == 0
    NT = bc // (P * G)

    nc = bass.Bass("TRN2")
    x = nc.dram_tensor("x", [bc, T, D], F32, kind="ExternalInput")
    wts = nc.dram_tensor("wts", [CLEN], F32, kind="ExternalInput")
    wtsb = nc.dram_tensor("wtsb", [CBLEN], BF16, kind="ExternalInput")
    out = nc.dram_tensor("out", [bc, T, D], F32, kind="ExternalOutput")

    xr = x[:].rearrange("(n g p) t d -> n p g t d", g=G, p=P)
    outr = out[:].rearrange("(n g p) t d -> n p g t d", g=G, p=P)
    wts_b = bass.AP(tensor=wts, offset=0, ap=[[0, P], [1, CLEN]])
    wtsb_b = bass.AP(tensor=wtsb, offset=0, ap=[[0, P], [1, CBLEN]])

    with ExitStack() as ctx:
        sb = lambda nm, shape, dt=F32: ctx.enter_context(
            nc.sbuf_tensor(nm, shape, dt))
        wsb = sb("wsb", [P, CLEN])
        wsbb = sb("wsbb", [P, CBLEN], BF16)
        xin = sb("xin", [P, 2, G, T, D])
        yv = sb("yv", [P, 3, 2, G, D, T], BF16)   # [n%3][w][g][hc][j]
        pp = sb("pp", [P, G, NH, T, T], BF16)
        t1 = sb("t1", [P, 2, G, NH, T, T], BF16)
        scr = sb("scr", [P, G, NH, T, T], BF16)   # vector-private
        xbc = sb("xbc", [P, POS, G, T, T], BF16)  # ACT-written xp planes
        den = sb("den", [P, G, NH, T])
        rcp = sb("rcp", [P, G, NH, T])
        o2 = sb("o2", [P, G, T, D])
        o2b = sb("o2b", [P, G, T, D], BF16)
        prod = sb("prod", [P, G, T, D, D], BF16)  # [(g,t)][dm][e]
        p3 = sb("p3", [P, G, T, D, HD], BF16)
        res = sb("res", [P, 2, G, T, D])
        pacc = sb("pacc", [P, G, D, T])           # gpsimd-private
        ptmp = sb("ptmp", [P, G, D, T])
        # bf16 fold-tree scratches (vector-private, reused per reduction)
        f17 = sb("f17", [P, G, NH, 17, 17], BF16)
        tw8 = sb("tw8", [P, G, NH, T, 8], BF16)
        tw4 = sb("tw4", [P, G, NH, T, 4], BF16)
        tw2 = sb("tw2", [P, G, NH, T, 2], BF16)
        tsum = sb("tsum", [P, G, NH, T], BF16)

        sem_names = ["dma_in0", "dma_in1", "const", "constb", "proj_done",
                     "a_done", "s_done", "e_done", "b_done", "res_done",
                     "d_done", "out0", "out1"]
        sems = {k: ctx.enter_context(nc.semaphore(name=k)) for k in sem_names}

        XIN_SET = G * T * D
        XIN_G = T * D
        YV_BUF = 2 * G * T * D
        YV_W = G * T * D
        YV_G = T * D
        TS_SET = G * NH * TT
        H = T // 2
        BLKS = [(0, 0), (H, 0), (H, H)]

        block = ctx.enter_context(nc.Block())

        @block.gpsimd
        def _(sync):
            def store(k):
                sp = k % 2
                sync.wait_ge(sems["res_done"], k + 1)
                sync.dma_start(
                    out=outr[k],
                    in_=_ap(res, sp * XIN_SET, [(XIN_G, G), (1, T * D)]),
                ).then_inc(sems["out0" if sp == 0 else "out1"], 16)

            def load(n):
                s = n % 2
                if n >= 2:
                    # xin[s] free once ACT's xbc(n-2) has read it
                    sync.wait_ge(sems["a_done"], n - 1)
                sync.dma_start(
                    out=_ap(xin, s * XIN_SET, [(XIN_G, G), (1, T * D)]),
                    in_=xr[n],
                ).then_inc(sems["dma_in0" if s == 0 else "dma_in1"], 16)

            def proj(n):
                s = n % 2
                m = n % 3
                sync.wait_ge(sems["dma_in0" if s == 0 else "dma_in1"],
                             16 * (n // 2 + 1))
                if n >= 3:
                    # WAR: yv[m] last read by PV muls of phase_b(n-3)
                    sync.wait_ge(sems["b_done"], n - 2)
                last = None
                for w in range(2):
                    xoff = s * XIN_SET + (3 - 3 * w)
                    for b in range(POS):
                        i0 = _ap(xin, xoff + b, [(XIN_G, G), (0, D), (D, T)])
                        i1 = _ap(wsb, OFF_A2 + w * 18 + b,
                                 [(0, G), (3, D), (0, T)])
                        if b == 0:
                            sync.tensor_mul(
                                out=_ap(pacc, 0, [(D * T, G), (T, D), (1, T)]),
                                in0=i0, in1=i1)
                        else:
                            sync.tensor_mul(
                                out=_ap(ptmp, 0, [(D * T, G), (T, D), (1, T)]),
                                in0=i0, in1=i1)
                            dst = (_ap(yv, m * YV_BUF + w * YV_W,
                                       [(YV_G, G), (1, T * D)]) if b == 2
                                   else _ap(pacc, 0, [(D * T, G), (1, T * D)]))
                            last = sync.tensor_add(
                                out=dst,
                                in0=_ap(pacc, 0, [(D * T, G), (1, T * D)]),
                                in1=_ap(ptmp, 0, [(D * T, G), (1, T * D)]))
                last.then_inc(sems["proj_done"], 1)

            sync.dma_start(out=wsb[:], in_=wts_b).then_inc(sems["const"], 16)
            sync.dma_start(out=wsbb[:], in_=wtsb_b).then_inc(
                sems["constb"], 16)
            load(0)
            load(1)
            sync.wait_ge(sems["const"], 16)
            for n in range(NT):
                if n == 0:
                    # tile 0's projection runs on vector (startup warmup);
                    # xin[0] also read there, so gate its reuse
                    sync.wait_ge(sems["d_done"], 1)
                else:
                    proj(n)
                if n >= 2:
                    store(n - 2)
                if n + 2 < NT:
                    load(n + 2)
            store(NT - 2)
            store(NT - 1)
            sync.wait_ge(sems["out0"], 16 * ((NT + 1) // 2))
            sync.wait_ge(sems["out1"], 16 * (NT // 2))

        @block.scalar
        def _(scalar):
            def xbc_fill(n):
                # xbc[a, g, i, j] = xp[g, i, a]  (bf16, j-packed), causal
                # blocks only: AB column (all rows, j<H) + C diag block.
                s = n % 2
                scalar.wait_ge(sems["dma_in0" if s == 0 else "dma_in1"],
                               16 * (n // 2 + 1))
                last = None
                for a in range(POS):
                    for (ro, io, li, lj) in ((0, 0, T, H),
                                             (H * T + H, H * D, T - H, T - H)):
                        last = scalar.activation(
                            out=_ap(xbc, a * G * TT + ro,
                                    [(TT, G), (T, li), (1, lj)]),
                            in_=_ap(xin, s * XIN_SET + 3 + a + io,
                                    [(XIN_G, G), (D, li), (0, lj)]),
                            func=mybir.ActivationFunctionType.Copy,
                        )
                last.then_inc(sems["a_done"], 1)

            xbc_fill(0)
            for n in range(NT):
                s = n % 2
                if n >= 2:
                    scalar.wait_ge(sems["b_done"], n - 1)
                scalar.wait_ge(sems["s_done"], n + 1)
                # exp in-place over causal blocks of t1[s]
                scalar.activation(
                    out=_ap(t1, s * TS_SET, [(TT, G * NH), (T, T), (1, H)]),
                    in_=_ap(t1, s * TS_SET, [(TT, G * NH), (T, T), (1, H)]),
                    func=mybir.ActivationFunctionType.Exp,
                )
                ro = H * T + H
                scalar.activation(
                    out=_ap(t1, s * TS_SET + ro,
                            [(TT, G * NH), (T, T - H), (1, T - H)]),
                    in_=_ap(t1, s * TS_SET + ro,
                            [(TT, G * NH), (T, T - H), (1, T - H)]),
                    func=mybir.ActivationFunctionType.Exp,
                ).then_inc(sems["e_done"], 1)
                if n + 1 < NT:
                    xbc_fill(n + 1)

        @block.vector
        def _(vector):
            vector.wait_ge(sems["constb"], 16)
            vector.wait_ge(sems["const"], 16)
            vector.wait_ge(sems["dma_in0"], 16)
            # tile-0 projection on vector (gpsimd's first proj would stall
            # the pipeline ~22us); fp32 scratch borrowed from res[1]/o2,
            # both first written much later
            last = None
            for w in range(2):
                xoff = 3 - 3 * w
                for b in range(POS):
                    i0 = _ap(xin, xoff + b, [(XIN_G, G), (0, D), (D, T)])
                    i1 = _ap(wsb, OFF_A2 + w * 18 + b,
                             [(0, G), (3, D), (0, T)])
                    if b == 0:
                        vector.tensor_mul(
                            out=_ap(res, XIN_SET,
                                    [(D * T, G), (T, D), (1, T)]),
                            in0=i0, in1=i1)
                    else:
                        vector.tensor_mul(
                            out=_ap(o2, 0, [(D * T, G), (T, D), (1, T)]),
                            in0=i0, in1=i1)
                        dst = (_ap(yv, w * YV_W, [(YV_G, G), (1, T * D)])
                               if b == 2
                               else _ap(res, XIN_SET,
                                        [(D * T, G), (1, T * D)]))
                        last = vector.tensor_add(
                            out=dst,
                            in0=_ap(res, XIN_SET, [(D * T, G), (1, T * D)]),
                            in1=_ap(o2, 0, [(D * T, G), (1, T * D)]))
            last.then_inc(sems["d_done"], 1)

            def phase_a(n):
                s = n % 2
                m = n % 3
                if n >= 1:
                    vector.wait_ge(sems["proj_done"], n)
                vector.wait_ge(sems["a_done"], n + 1)
                # scores: t1[g,h,i,j] = sum_a xbc[a,g,i,j] * y[g,(h,a),j]
                # two merged causal regions: AB column (all rows, j<H) and
                # C diag block
                for a in range(POS):
                    dst = t1 if a == 0 else scr
                    doff = s * TS_SET if a == 0 else 0
                    for h in range(NH):
                        for (ro, li) in ((0, T), (H * T + H, T - H)):
                            vector.tensor_mul(
                                out=_ap(dst, doff + h * TT + ro,
                                        [(NH * TT, G), (T, li), (1, H)]),
                                in0=_ap(xbc, a * G * TT + ro,
                                        [(TT, G), (T, li), (1, H)]),
                                in1=_ap(yv, m * YV_BUF + (h * HD + a) * T
                                        + (0 if ro == 0 else H),
                                        [(YV_G, G), (0, li), (1, H)]))
                    if a == 2:
                        # scr += mask on diagonal blocks A and C
                        for ro in (0, H * T + H):
                            vector.tensor_add(
                                out=_ap(scr, ro, [(TT, G * NH), (T, H), (1, H)]),
                                in0=_ap(scr, ro, [(TT, G * NH), (T, H), (1, H)]),
                                in1=_ap(wsbb, OFF_MASK + ro,
                                        [(0, G * NH), (T, H), (1, H)]))
                    if a > 0:
                        # t1 += scr over AB column + C block
                        mm = None
                        for (ro, li, lj) in ((0, T, H), (H * T + H, T - H, T - H)):
                            mm = vector.tensor_add(
                                out=_ap(t1, s * TS_SET + ro,
                                        [(TT, G * NH), (T, li), (1, lj)]),
                                in0=_ap(t1, s * TS_SET + ro,
                                        [(TT, G * NH), (T, li), (1, lj)]),
                                in1=_ap(scr, ro,
                                        [(TT, G * NH), (T, li), (1, lj)]))
                        if a == 2:
                            mm.then_inc(sems["s_done"], 1)

            GH = G * NH

            def fold_tree(src, soff):
                """bf16 pairwise row-sum tree over causal planes of src.

                After this: tsum[g,h,i] + leftover col 16 (A rows: src[i,16],
                BC rows: f17[i-17,16]) equals the causal row sum."""
                # L1: rows 17-33: fold j and j+17
                vector.tensor_add(
                    out=_ap(f17, 0, [(289, GH), (17, 17), (1, 17)]),
                    in0=_ap(src, soff + H * T, [(TT, GH), (T, 17), (1, 17)]),
                    in1=_ap(src, soff + H * T + H,
                            [(TT, GH), (T, 17), (1, 17)]))
                # L2a: A rows (0-16): j pairs (0-7)+(8-15)
                vector.tensor_add(
                    out=_ap(tw8, 0, [(T * 8, GH), (8, 17), (1, 8)]),
                    in0=_ap(src, soff, [(TT, GH), (T, 17), (1, 8)]),
                    in1=_ap(src, soff + 8, [(TT, GH), (T, 17), (1, 8)]))
                # L2f: folded rows -> tw8 rows 17-33
                vector.tensor_add(
                    out=_ap(tw8, 17 * 8, [(T * 8, GH), (8, 17), (1, 8)]),
                    in0=_ap(f17, 0, [(289, GH), (17, 17), (1, 8)]),
                    in1=_ap(f17, 8, [(289, GH), (17, 17), (1, 8)]))
                # L3: 8->4, L4: 4->2
                vector.tensor_add(
                    out=_ap(tw4, 0, [(T * 4, GH), (4, T), (1, 4)]),
                    in0=_ap(tw8, 0, [(T * 8, GH), (8, T), (1, 4)]),
                    in1=_ap(tw8, 4, [(T * 8, GH), (8, T), (1, 4)]))
                vector.tensor_add(
                    out=_ap(tw2, 0, [(T * 2, GH), (2, T), (1, 2)]),
                    in0=_ap(tw4, 0, [(T * 4, GH), (4, T), (1, 2)]),
                    in1=_ap(tw4, 2, [(T * 4, GH), (4, T), (1, 2)]))
                # L5: 2->1
                vector.tensor_add(
                    out=_ap(tsum, 0, [(T, GH), (1, T)]),
                    in0=_ap(tw2, 0, [(T * 2, GH), (2, T)]),
                    in1=_ap(tw2, 1, [(T * 2, GH), (2, T)]))

            def phase_b(n):
                s = n % 2
                m = n % 3
                vector.wait_ge(sems["e_done"], n + 1)
                if n >= 2:
                    vector.wait_ge(sems["out0" if s == 0 else "out1"],
                                   16 * (n // 2))
                # den: fold tree + leftover col, fp32 out
                fold_tree(t1, s * TS_SET)
                vector.tensor_add(
                    out=_ap(den, 0, [(T, GH), (1, 17)]),
                    in0=_ap(tsum, 0, [(T, GH), (1, 17)]),
                    in1=_ap(t1, s * TS_SET + 16, [(TT, GH), (T, 17)]))
                vector.tensor_add(
                    out=_ap(den, H, [(T, GH), (1, 17)]),
                    in0=_ap(tsum, 17, [(T, GH), (1, 17)]),
                    in1=_ap(f17, 16, [(289, GH), (17, 17)]))
                vector.reciprocal(
                    out=_ap(rcp, 0, [(1, G * NH * T)]),
                    in_=_ap(den, 0, [(1, G * NH * T)]))
                for c in range(HD):
                    for h in range(NH):
                        for bi, (ro, li) in enumerate(((0, T),
                                                      (H * T + H, T - H))):
                            mm = vector.tensor_mul(
                                out=_ap(pp, h * TT + ro,
                                        [(NH * TT, G), (T, li), (1, H)]),
                                in0=_ap(t1, s * TS_SET + h * TT + ro,
                                        [(NH * TT, G), (T, li), (1, H)]),
                                in1=_ap(yv, m * YV_BUF + YV_W + (h * HD + c) * T
                                        + (0 if ro == 0 else H),
                                        [(YV_G, G), (0, li), (1, H)]))
                            if c == HD - 1 and h == NH - 1 and bi == 1:
                                mm.then_inc(sems["b_done"], 1)
                    fold_tree(pp, 0)
                    vector.tensor_add(
                        out=_ap(o2, c, [(T * D, G), (HD, NH), (D, 17)]),
                        in0=_ap(tsum, 0, [(NH * T, G), (T, NH), (1, 17)]),
                        in1=_ap(pp, 16, [(NH * TT, G), (TT, NH), (T, 17)]))
                    vector.tensor_add(
                        out=_ap(o2, c + H * D, [(T * D, G), (HD, NH), (D, 17)]),
                        in0=_ap(tsum, 17, [(NH * T, G), (T, NH), (1, 17)]),
                        in1=_ap(f17, 16, [(NH * 289, G), (289, NH), (17, 17)]))
                for h in range(NH):
                    vector.tensor_mul(
                        out=_ap(o2b, h * HD, [(T * D, G), (D, T), (1, HD)]),
                        in0=_ap(o2, h * HD, [(T * D, G), (D, T), (1, HD)]),
                        in1=_ap(rcp, h * T, [(NH * T, G), (1, T), (0, HD)]))
                # outproj: prod[(g,t),dm,e] = o2b[g,t,e] * Wo[dm,e], fold
                # e-halves, reduce remaining 3
                vector.tensor_mul(
                    out=_ap(prod, 0, [(D * D, G * T), (D, D), (1, D)]),
                    in0=_ap(o2b, 0, [(D, G * T), (0, D), (1, D)]),
                    in1=_ap(wsbb, OFF_WO, [(0, G * T), (D, D), (1, D)]))
                vector.tensor_add(
                    out=_ap(p3, 0, [(D * HD, G * T), (HD, D), (1, HD)]),
                    in0=_ap(prod, 0, [(D * D, G * T), (D, D), (1, HD)]),
                    in1=_ap(prod, HD, [(D * D, G * T), (D, D), (1, HD)]))
                vector.tensor_add(
                    out=_ap(o2b, 0, [(D, G * T), (1, D)]),
                    in0=_ap(p3, 0, [(D * HD, G * T), (HD, D)]),
                    in1=_ap(p3, 1, [(D * HD, G * T), (HD, D)]))
                vector.tensor_add(
                    out=_ap(res, s * XIN_SET, [(D, G * T), (1, D)]),
                    in0=_ap(o2b, 0, [(D, G * T), (1, D)]),
                    in1=_ap(p3, 2, [(D * HD, G * T), (HD, D)])
                ).then_inc(sems["res_done"], 1)

            for n in range(NT):
                phase_a(n)
                if n >= 1:
                    phase_b(n - 1)
            phase_b(NT - 1)

    return nc


def _pack_weights(Wq, Wk, Wv, Wo):
    from ml_dtypes import bfloat16
    wts = np.zeros(CLEN, dtype=np.float32)
    scale = 1.0 / math.sqrt(HD)
    A2 = wts[OFF_A2:OFF_A2 + 36].reshape(2, D, POS)
    for h in range(NH):
        A2[0, h * HD:(h + 1) * HD, :] = (Wq[h * HD:(h + 1) * HD, :].T
                                         @ Wk[h * HD:(h + 1) * HD, :]) * scale
        A2[1, h * HD:(h + 1) * HD, :] = Wv[h * HD:(h + 1) * HD, :]
    wtsb = np.zeros(CBLEN, dtype=np.float32)
    mask = np.where(np.tril(np.ones((T, T))) > 0, 0.0, -1e9).astype(np.float32)
    wtsb[OFF_MASK:OFF_MASK + TT] = mask.reshape(-1)
    wtsb[OFF_WO:OFF_WO + 36] = Wo.reshape(-1)
    return wts, wtsb.astype(bfloat16)


@lru_cache(maxsize=2)
def _cached_kernel(bc, G):
    return build_kernel(bc, G)


def kernel(x, Wq, Wk, Wv, Wo):
    x = np.ascontiguousarray(x, dtype=np.float32)
    B = x.shape[0]
    bc = B // NCORES
    G = 4
    nc = _cached_kernel(bc, G)
    wts, wtsb = _pack_weights(np.asarray(Wq, dtype=np.float32),
                              np.asarray(Wk, dtype=np.float32),
                              np.asarray(Wv, dtype=np.float32),
                              np.asarray(Wo, dtype=np.float32))
    in_maps = [{"x": x[i * bc:(i + 1) * bc], "wts": wts, "wtsb": wtsb}
               for i in range(NCORES)]
    r = run_bass_kernel_spmd(nc, in_maps, core_ids=list(range(NCORES)))
    return np.concatenate([m["out"] for m in r.results], axis=0)
